# revision 2
# baseline (speedup 1.0000x reference)
"""Trainium2 Bass kernel for nn_MultiHeadAttention_70866960384614.

MHA: B=2, S=2048, D_MODEL=1024, HEADS=16, D_K=64, softmax(|QK^T|/8) @ V.

Sharding (8 cores): data-parallel over batch (2) x tensor-parallel over head
groups (4 groups of 4 heads). Host pre-transposes x and the weight slices so
the device does ZERO layout transposes: per core the inputs are
  xqT/xkT/xvT [1024, 2048]  (x^T, fp32)
  wqT/wkT/wvT [1024, 256]   (W[group].T)
  woT         [256, 1024]   (Wo[:, group].T)
DMA feeds matmul operands directly as float32r (full-rate fp32 on the PE).

Per-core dataflow:
  QT = Wq @ xT (+bq)            [256, 2048]   (= wqT.T @ xqT)
  KT likewise; V = x @ WvT + bv  stored kv-major with a ones column per head
  per (qchunk, head-pair): for each kv pair:
     ST = K Q^T (2 heads packed in the PE array), DVE/ACT |.| exit to SBUF,
     one bulk ACT exp over [128, 2048], PV accumulates [V|1].T @ P
  normalization: Z rows -> DVE reciprocal -> PE broadcast -> DVE multiply
  out partial = catT.T @ WoT -> DMA (host sums the 4 group partials + bo)
"""

import json
from contextlib import ExitStack

import numpy as np

import concourse.bass as bass
import concourse.mybir as mybir
import concourse.tile as tile
from concourse.vector_clock import ScopedClock

F32 = mybir.dt.float32
F32R = mybir.dt.float32r
BF16 = mybir.dt.bfloat16
AF = mybir.ActivationFunctionType
ALU = mybir.AluOpType

S = 2048
D = 1024
HG = 4            # heads per core
DK = 64
GC = HG * DK      # 256
P = 128
N_CORES = 8
SC = 512          # s-chunk for projection streaming
QC = 512          # q-chunk within attention
KVB = S // P      # 16 kv blocks
SCALE = 0.125

# every ACT_EXIT_EVERYth score-exit goes to ACT instead of DVE (engine balance)
ACT_EXIT_EVERY = 0   # 0 = never


class TileContextCompat(tile.TileContext):
    """This container's walrus build rejects >1 sync-wait on a CTRL (Drain)
    instruction; spread the kernel-tail DMA-lane waits across one drain
    each instead of piling them on a single drain."""

    def _drain_and_barrier(self, tick_clock, wait_clock):
        drain_inst = self.nc.sync.drain()
        wait_clock.add_sem_waits(
            drain_inst.ins, ScopedClock({None: tick_clock.global_clock}))
        si = drain_inst.ins.sync_info
        extra = []
        if si is not None and si.on_wait is not None:
            while len(si.on_wait) > 1:
                extra.append(si.on_wait.pop())
        for w in extra:
            d2 = self.nc.sync.drain()
            if d2.ins.sync_info is None:
                d2.ins.sync_info = mybir.SyncInfo(on_wait=[w], on_update=[])
            else:
                d2.ins.sync_info.on_wait.append(w)
        self.nc.all_engine_barrier()
        assert self.sems is not None
        popped = self.nc._tile_sem_poison_stack.pop()
        assert popped is self._sem_poison
        self.nc.clear_and_free_semaphores(list(self.sems.allocated().values()))
        self.nc.all_engine_barrier()


def build_nc():
    nc = bass.Bass("TRN2", target_bir_lowering=False, debug=False,
                   num_devices=N_CORES)

    xqt = nc.dram_tensor("xqt", [D, S], BF16, kind="ExternalInput").ap()
    xkt = nc.dram_tensor("xkt", [D, S], BF16, kind="ExternalInput").ap()
    xvt = nc.dram_tensor("xvt", [D, S], BF16, kind="ExternalInput").ap()
    wqt = nc.dram_tensor("wqt", [D, GC], BF16, kind="ExternalInput").ap()
    wkt = nc.dram_tensor("wkt", [D, GC], BF16, kind="ExternalInput").ap()
    wvt = nc.dram_tensor("wvt", [D, GC], BF16, kind="ExternalInput").ap()
    wot = nc.dram_tensor("wot", [GC, D], F32, kind="ExternalInput").ap()
    bq = nc.dram_tensor("bq", [GC], BF16, kind="ExternalInput").ap()
    bk = nc.dram_tensor("bk", [GC], BF16, kind="ExternalInput").ap()
    bv = nc.dram_tensor("bv", [GC], BF16, kind="ExternalInput").ap()
    out = nc.dram_tensor("out", [S, D], F32, kind="ExternalOutput").ap()

    with ExitStack() as ctx:
        tc = ctx.enter_context(TileContextCompat(nc))
        _emit(ctx, tc, xqt, xkt, xvt, wqt, wkt, wvt, wot, bq, bk, bv, out)

    fixed = _split_multi_waits(nc.to_json_bytes())
    nc.to_json_bytes = lambda: fixed
    return nc


def _split_multi_waits(raw):
    """Walrus here accepts only one sync-wait per instruction; hoist extras
    onto wait-only EventSemaphore instructions on the same engine."""
    m = json.loads(raw)
    counter = [0]

    def fix_block(b):
        new = []
        for inst in b.get("instructions", []):
            si = inst.get("sync_info")
            if si and si.get("on_wait") and len(si["on_wait"]) > 1:
                waits = si["on_wait"]
                for w in waits[:-1]:
                    counter[0] += 1
                    new.append({
                        "debug": inst.get("debug", 0),
                        "engine": inst["engine"],
                        "ins": [],
                        "outs": [],
                        "name": f"I-wsplit-{counter[0]}",
                        "opcode": "EventSemaphore",
                        "sync_info": {"on_update": [], "on_wait": [w]},
                    })
                si["on_wait"] = waits[-1:]
            new.append(inst)
        b["instructions"] = new
        for sub in b.get("blocks", []):
            fix_block(sub)

    for fn in m["functions"]:
        for b in fn.get("blocks", []):
            fix_block(b)
    return json.dumps(m).encode()


def _emit(ctx, tc, xqt, xkt, xvt, wqt, wkt, wvt, wot, bq, bk, bv, out):
    nc = tc.nc

    persist = ctx.enter_context(tc.tile_pool(name="persist", bufs=1))
    xs = ctx.enter_context(tc.tile_pool(name="xs", bufs=3))
    stp = ctx.enter_context(tc.tile_pool(name="st", bufs=2, space="PSUM"))
    opp = ctx.enter_context(tc.tile_pool(name="op", bufs=2, space="PSUM"))
    pap = ctx.enter_context(tc.tile_pool(name="pa", bufs=3))
    sap = ctx.enter_context(tc.tile_pool(name="sa", bufs=3))
    zp = ctx.enter_context(tc.tile_pool(name="zp", bufs=2))
    otp = ctx.enter_context(tc.tile_pool(name="ot", bufs=2))
    catp = ctx.enter_context(tc.tile_pool(name="cat", bufs=2))

    # ------------------------------------------------------- persistent
    qT = persist.tile([P, 2, S], F32R)
    kT = persist.tile([P, 2, S], F32R)
    vA = persist.tile([P, KVB, HG * (DK + 1)], F32R)
    wq_s = persist.tile([P, D // P, GC], BF16)
    wk_s = persist.tile([P, D // P, GC], BF16)
    wv_s = persist.tile([P, D // P, GC], BF16)
    wo_s = persist.tile([P, GC // P, D], F32R)
    bq_r = persist.tile([1, GC], BF16)
    bk_r = persist.tile([1, GC], BF16)
    bv_r = persist.tile([1, GC], BF16)

    # ones rows (memset + DVE self-copy so they count as f32r-produced)
    ones_row = persist.tile([1, QC], BF16)
    nc.vector.memset(ones_row, 1.0)
    ones_bcr = persist.tile([1, DK], F32)
    nc.vector.memset(ones_bcr, 1.0)
    # V ones columns
    nc.vector.memset(vA.bitcast(F32), 1.0)
    ones_cols = vA[:].rearrange("p s (h c) -> p s h c", h=HG)[:, :, :, DK:]
    nc.vector.tensor_copy(ones_cols, ones_cols.bitcast(F32))

    # ------------------------------------------------------ weight DMAs
    # (K and Q weights first: the first ST pair needs only kT/qT chunk 0;
    # V weights and chunks follow just in time for the lagged PV matmuls.)
    def dma_w(w_dram, w_sb):
        nc.sync.dma_start(
            w_sb, w_dram.rearrange("(kc p) c -> p kc c", p=P))

    dma_w(wkt, wk_s)
    dma_w(wqt, wq_s)
    nc.gpsimd.dma_start(bq_r, bq[None, :])
    nc.gpsimd.dma_start(bk_r, bk[None, :])
    nc.gpsimd.dma_start(bv_r, bv[None, :])

    # PE warmup: a continuous chain of tiny matmuls spans the initial DMA
    # window so the PE p-state is fully ramped when the projections start.
    wup = stp.tile([P, 2 * QC], F32, tag="st", name="wup")
    for _ in range(250):
        nc.tensor.matmul(wup[0:1, 0:DK], ones_row[0:1, 0:1],
                         ones_row[0:1, 0:DK], start=True, stop=True)

    exit_ctr = [0]

    def score_exit(dst, src):
        exit_ctr[0] += 1
        if ACT_EXIT_EVERY and exit_ctr[0] % ACT_EXIT_EVERY == 0:
            nc.scalar.activation(dst, src, AF.Abs)
        else:
            du, su = dst.bitcast(mybir.dt.uint32), src.bitcast(mybir.dt.uint32)
            nc.vector.tensor_scalar(du, su, 0x7FFFFFFF, None, ALU.bitwise_and)

    def proj_dma(x_dram, sc):
        xt = xs.tile([P, D // P, SC], BF16, tag="xs")
        nc.sync.dma_start(
            xt, x_dram.rearrange("(kc p) s -> p kc s", p=P)
            [:, :, sc * SC:(sc + 1) * SC])
        return xt

    def proj_mm(xt, sc, which):
        """Matmuls+exit for one SC-chunk of a projection. which: 'k'|'v'|'q'."""
        ps = stp.tile([P, 2 * QC], F32, tag="st", name="pj")
        if which in ("k", "q"):
            w_sb, dstT, b_r = ((wk_s, kT, bk_r) if which == "k"
                               else (wq_s, qT, bq_r))
            for m in range(2):
                half = ps[:, m * SC:(m + 1) * SC]
                for kc in range(D // P):
                    nc.tensor.matmul(
                        half, w_sb[:, kc, m * P:(m + 1) * P],
                        xt[:, kc, :], start=(kc == 0), stop=False)
                nc.tensor.matmul(
                    half, b_r[0:1, m * P:(m + 1) * P],
                    ones_row, start=False, stop=True)
            dst = dstT[:, :, sc * SC:(sc + 1) * SC]
            nc.scalar.activation(
                dst, ps[:].rearrange("p (m f) -> p m f", m=2), AF.Copy)
        else:
            for sb in range(SC // P):
                seg = ps[:, sb * GC:(sb + 1) * GC]
                for kc in range(D // P):
                    nc.tensor.matmul(
                        seg, xt[:, kc, sb * P:(sb + 1) * P],
                        wv_s[:, kc, :], start=(kc == 0), stop=False)
                nc.tensor.matmul(seg, ones_row[0:1, 0:P],
                                 bv_r, start=False, stop=True)
            gsb = sc * (SC // P)
            dst = vA[:, gsb:gsb + 4, :].rearrange(
                "p s (h c) -> p s h c", h=HG)[:, :, :, :DK]
            src = ps[:].rearrange("p (s h c) -> p s h c", s=4, h=HG)
            nc.scalar.activation(dst, src, AF.Copy)

    def proj_chunk(x_dram, sc, which):
        proj_mm(proj_dma(x_dram, sc), sc, which)

    # K, Q chunk 0 first (unblocks the first ST pair), then V weights +
    # chunk 0 (needed one pair later by the lagged PVs), then Wo. The
    # remaining K/V chunks interleave into the first attention block.
    proj_chunk(xkt, 0, "k")
    proj_chunk(xqt, 0, "q")
    dma_w(wvt, wv_s)
    proj_chunk(xvt, 0, "v")

    # ---------------------------------------------------- attention
    # Two levels of software pipelining against the in-order PE queue:
    #  - PV matmuls run one kv-pair behind their STs (never wait on abs/exp)
    #  - each block's normalization/out-projection tail is deferred into the
    #    middle of the NEXT block, so block boundaries don't stall DVE/ACT.
    pending_pv = []
    pending_tail = []

    def flush_pv():
        for mm in pending_pv:
            mm()
        pending_pv.clear()

    def flush_tail(n=None):
        take = len(pending_tail) if n is None else min(n, len(pending_tail))
        for f in pending_tail[:take]:
            f()
        del pending_tail[:take]

    cat_tiles = {}

    def get_cat(qc):
        if qc not in cat_tiles:
            cat_tiles[qc] = catp.tile([P, 2, QC], F32R, tag="cat", name=f"cat{qc}")
        return cat_tiles[qc]

    def norm_a(qc, pr, opt):
        catT = get_cat(qc)
        # 1/Z via exp(-ln(Z)) on ACT: certified f32r writes, PSUM-direct read
        rec = zp.tile([1, 2 * QC], F32, tag="rec")
        nc.scalar.activation(
            rec, opt[:].rearrange("p (h f) -> p h f", h=2)[64:65, :, :], AF.Ln)
        nc.scalar.activation(rec, rec, AF.Exp, scale=-1.0)
        for half in range(2):
            csl = slice(half * DK, (half + 1) * DK)
            dst = catT[csl, pr, :]
            nc.scalar.activation(
                dst, opt[0:DK, half * QC:(half + 1) * QC], AF.Copy)
        return rec

    def norm_b(qc, pr, rec):
        catT = get_cat(qc)
        bc2 = opp.tile([P, 2 * QC], F32, tag="o", name="bc")
        for half in range(2):
            csl = slice(half * DK, (half + 1) * DK)
            bc = bc2[csl, half * QC:(half + 1) * QC]
            nc.tensor.matmul(bc, ones_bcr,
                             rec[0:1, half * QC:(half + 1) * QC],
                             start=True, stop=True,
                             tile_position=(0, half * DK))
            dst = catT[csl, pr, :]
            nc.vector.tensor_tensor(dst, dst, bc.bitcast(F32R), ALU.mult)

    def norm(qc, pr, opt):
        norm_b(qc, pr, norm_a(qc, pr, opt))

    def outproj(qc, js):
        catT = get_cat(qc)
        for j in js:
            sb = qc * (QC // P) + j
            o_t = otp.tile([P, D], F32, tag="ot")
            po = stp.tile([P, 2 * QC], F32, tag="st", name="po")
            for nn in range(2):
                seg = po[:, nn * QC:(nn + 1) * QC]
                for kc in range(2):
                    nc.tensor.matmul(
                        seg, catT[:, kc, j * P:(j + 1) * P],
                        wo_s[:, kc, nn * QC:(nn + 1) * QC],
                        start=(kc == 0), stop=(kc == 1))
            nc.scalar.activation(o_t, po, AF.Copy)
            nc.sync.dma_start(out[sb * P:(sb + 1) * P, :], o_t)

    def attn_pair(qc, pr, pv, opt):
        qsl = slice(qc * QC, (qc + 1) * QC)
        hA, hB = 2 * pr, 2 * pr + 1
        pa_t = pap.tile([P, 2 * 2 * QC], F32R, tag="pa")
        sabs = sap.tile([P, 2 * 2 * QC], F32, tag="sa")
        for i in range(2):
            kv = 2 * pv + i
            ksl = slice(kv * P, (kv + 1) * P)
            st = stp.tile([P, 2 * QC], F32, tag="st", name="stt")
            nc.tensor.matmul(
                st[:, :QC], kT[0:DK, pr, ksl],
                qT[0:DK, pr, qsl], start=True, stop=True,
                tile_position=(0, 0))
            nc.tensor.matmul(
                st[:, QC:], kT[DK:P, pr, ksl],
                qT[DK:P, pr, qsl], start=True, stop=True,
                tile_position=(DK, 0))
            score_exit(sabs[:, i * 2 * QC:(i + 1) * 2 * QC], st)
        flush_pv()
        nc.scalar.activation(pa_t, sabs, AF.Exp, scale=SCALE)

        def mk_pv():
            for i in range(2):
                kv = 2 * pv + i
                off = i * 2 * QC
                nc.tensor.matmul(
                    opt[0:DK + 1, :QC],
                    vA[:, kv, hA * 65:hA * 65 + 65],
                    pa_t[:, off:off + QC],
                    start=(kv == 0), stop=(kv == KVB - 1))
                nc.tensor.matmul(
                    opt[0:DK + 1, QC:],
                    vA[:, kv, hB * 65:hB * 65 + 65],
                    pa_t[:, off + QC:off + 2 * QC],
                    start=(kv == 0), stop=(kv == KVB - 1))
        pending_pv.append(mk_pv)

    # --- qc 0: pr0/pr1 pair streams merged so both head-pairs' abs/exp work
    # rides each arriving K/V chunk (DMA otherwise starves DVE/ACT here).
    opt0 = opp.tile([P, 2 * QC], F32, tag="o")
    opt1 = opp.tile([P, 2 * QC], F32, tag="o")
    kv_tiles = {}
    xq1_tile = None
    for pv in range(KVB // 2):
        # prefetch chunk DMAs one pair ahead of their matmuls (xs bufs=3)
        if pv in (0, 2, 4):
            sc = pv // 2 + 1
            kv_tiles[sc] = [proj_dma(xkt, sc)]
        if pv in (1, 3, 5):
            sc = (pv + 1) // 2
            kv_tiles[sc].append(proj_dma(xvt, sc))
        if pv in (2, 4, 6):
            sc = pv // 2
            flush_pv()
            xk_t, xv_t = kv_tiles.pop(sc)
            proj_mm(xk_t, sc, "k")
            proj_mm(xv_t, sc, "v")
        attn_pair(0, 0, pv, opt0)
        if pv == KVB // 2 - 1:
            flush_pv()
            norm(0, 0, opt0)
        attn_pair(0, 1, pv, opt1)
        if pv == 5:
            nc.sync.dma_start(
                wo_s, wot.bitcast(F32R).rearrange("(kc p) d -> p kc d", p=P))
        if pv == 6:
            xq1_tile = proj_dma(xqt, 1)
        if pv == 7:
            proj_mm(xq1_tile, 1, "q")
    flush_pv()
    pending_tail.append(lambda: norm(0, 1, opt1))
    pending_tail.append(lambda: outproj(0, (0, 1)))
    pending_tail.append(lambda: outproj(0, (2, 3)))

    # --- qc 1..3: per-(qc, pr) blocks with deferred tails
    for qc in range(1, S // QC):
        for pr in range(2):
            opt = opp.tile([P, 2 * QC], F32, tag="o")
            for pv in range(KVB // 2):
                attn_pair(qc, pr, pv, opt)
                if pv >= 1:
                    flush_tail(1)
            if pr == 0:
                box = {}

                def t_na(qc=qc, pr=pr, opt=opt, box=box):
                    box["rec"] = norm_a(qc, pr, opt)

                def t_nb(qc=qc, pr=pr, box=box):
                    norm_b(qc, pr, box["rec"])
                pending_tail.append(t_na)
                pending_tail.append(t_nb)
                if qc + 1 < S // QC:
                    pending_tail.append(
                        lambda qc=qc: proj_chunk(xqt, qc + 1, "q"))
            else:
                box = {}

                def t_na(qc=qc, pr=pr, opt=opt, box=box):
                    box["rec"] = norm_a(qc, pr, opt)

                def t_nb(qc=qc, pr=pr, box=box):
                    norm_b(qc, pr, box["rec"])
                pending_tail.append(t_na)
                pending_tail.append(t_nb)
                for j in range(4):
                    pending_tail.append(lambda qc=qc, j=j: outproj(qc, (j,)))
    flush_pv()
    flush_tail()


_NC_CACHE = {}


def _get_nc():
    if "nc" not in _NC_CACHE:
        _NC_CACHE["nc"] = build_nc()
    return _NC_CACHE["nc"]


def make_in_maps(q, k, v, Wq, bq, Wk, bk, Wv, bv, Wo, bo):
    import ml_dtypes
    bf16 = ml_dtypes.bfloat16
    xT = [np.ascontiguousarray(np.asarray(a, np.float32).T.astype(bf16))
          for a in (q[0], k[0], v[0], q[1], k[1], v[1])]
    in_maps = []
    for c in range(N_CORES):
        b, g = divmod(c, 4)
        sl = slice(g * GC, (g + 1) * GC)
        in_maps.append({
            "xqt": xT[3 * b + 0],
            "xkt": xT[3 * b + 1],
            "xvt": xT[3 * b + 2],
            "wqt": np.ascontiguousarray(Wq[sl].T.astype(bf16)),
            "wkt": np.ascontiguousarray(Wk[sl].T.astype(bf16)),
            "wvt": np.ascontiguousarray(Wv[sl].T.astype(bf16)),
            "wot": np.ascontiguousarray(Wo[:, sl].T),
            "bq": np.ascontiguousarray(bq[sl].astype(bf16)),
            "bk": np.ascontiguousarray(bk[sl].astype(bf16)),
            "bv": np.ascontiguousarray(bv[sl].astype(bf16)),
        })
    return in_maps


def kernel(q, k, v, Wq, bq, Wk, bk, Wv, bv, Wo, bo, _trace=False):
    from concourse.bass_utils import run_bass_kernel_spmd

    q, k, v = (np.asarray(a, np.float32) for a in (q, k, v))
    Wq, bq, Wk, bk, Wv, bv, Wo, bo = (
        np.asarray(a, np.float32) for a in (Wq, bq, Wk, bk, Wv, bv, Wo, bo))

    nc = _get_nc()
    in_maps = make_in_maps(q, k, v, Wq, bq, Wk, bk, Wv, bv, Wo, bo)
    res = run_bass_kernel_spmd(nc, in_maps, core_ids=list(range(N_CORES)),
                               trace=_trace)
    partials = np.stack([r["out"] for r in res.results])  # [8, S, D]
    full = partials.reshape(2, 4, S, D).sum(axis=1) + bo[None, None, :]
    if _trace:
        return full.astype(np.float32), res
    return full.astype(np.float32)


# revision 3
# speedup vs baseline: 1.0222x; 1.0222x over previous
"""Trainium2 Bass kernel for nn_MultiHeadAttention_70866960384614.

MHA: B=2, S=2048, D_MODEL=1024, HEADS=16, D_K=64, softmax(|QK^T|/8) @ V.

Sharding (8 cores): data-parallel over batch (2) x tensor-parallel over head
groups (4 groups of 4 heads). Host pre-transposes x and the weight slices so
the device does ZERO layout transposes: per core the inputs are
  xqT/xkT/xvT [1024, 2048]  (x^T, fp32)
  wqT/wkT/wvT [1024, 256]   (W[group].T)
  woT         [256, 1024]   (Wo[:, group].T)
DMA feeds matmul operands directly as float32r (full-rate fp32 on the PE).

Per-core dataflow:
  QT = Wq @ xT (+bq)            [256, 2048]   (= wqT.T @ xqT)
  KT likewise; V = x @ WvT + bv  stored kv-major with a ones column per head
  per (qchunk, head-pair): for each kv pair:
     ST = K Q^T (2 heads packed in the PE array), DVE/ACT |.| exit to SBUF,
     one bulk ACT exp over [128, 2048], PV accumulates [V|1].T @ P
  normalization: Z rows -> DVE reciprocal -> PE broadcast -> DVE multiply
  out partial = catT.T @ WoT -> DMA (host sums the 4 group partials + bo)
"""

import json
from contextlib import ExitStack

import numpy as np

import concourse.bass as bass
import concourse.mybir as mybir
import concourse.tile as tile
from concourse.vector_clock import ScopedClock

F32 = mybir.dt.float32
F32R = mybir.dt.float32r
BF16 = mybir.dt.bfloat16
AF = mybir.ActivationFunctionType
ALU = mybir.AluOpType

S = 2048
D = 1024
HG = 4            # heads per core
DK = 64
GC = HG * DK      # 256
P = 128
N_CORES = 8
SC = 512          # s-chunk for projection streaming
QC = 512          # q-chunk within attention
KVB = S // P      # 16 kv blocks
SCALE = 0.125

# every ACT_EXIT_EVERYth score-exit goes to ACT instead of DVE (engine balance)
ACT_EXIT_EVERY = 0   # 0 = never


class TileContextCompat(tile.TileContext):
    """This container's walrus build rejects >1 sync-wait on a CTRL (Drain)
    instruction; spread the kernel-tail DMA-lane waits across one drain
    each instead of piling them on a single drain."""

    def _drain_and_barrier(self, tick_clock, wait_clock):
        drain_inst = self.nc.sync.drain()
        wait_clock.add_sem_waits(
            drain_inst.ins, ScopedClock({None: tick_clock.global_clock}))
        si = drain_inst.ins.sync_info
        extra = []
        if si is not None and si.on_wait is not None:
            while len(si.on_wait) > 1:
                extra.append(si.on_wait.pop())
        for w in extra:
            d2 = self.nc.sync.drain()
            if d2.ins.sync_info is None:
                d2.ins.sync_info = mybir.SyncInfo(on_wait=[w], on_update=[])
            else:
                d2.ins.sync_info.on_wait.append(w)
        self.nc.all_engine_barrier()
        assert self.sems is not None
        popped = self.nc._tile_sem_poison_stack.pop()
        assert popped is self._sem_poison
        self.nc.clear_and_free_semaphores(list(self.sems.allocated().values()))
        self.nc.all_engine_barrier()


def build_nc():
    nc = bass.Bass("TRN2", target_bir_lowering=False, debug=False,
                   num_devices=N_CORES)

    xqt = nc.dram_tensor("xqt", [D, S], BF16, kind="ExternalInput").ap()
    xkt = nc.dram_tensor("xkt", [D, S], BF16, kind="ExternalInput").ap()
    xvt = nc.dram_tensor("xvt", [D, S], BF16, kind="ExternalInput").ap()
    wqt = nc.dram_tensor("wqt", [D, GC], BF16, kind="ExternalInput").ap()
    wkt = nc.dram_tensor("wkt", [D, GC], BF16, kind="ExternalInput").ap()
    wvt = nc.dram_tensor("wvt", [D, GC], BF16, kind="ExternalInput").ap()
    wot = nc.dram_tensor("wot", [GC, D], F32, kind="ExternalInput").ap()
    bq = nc.dram_tensor("bq", [GC], BF16, kind="ExternalInput").ap()
    bk = nc.dram_tensor("bk", [GC], BF16, kind="ExternalInput").ap()
    bv = nc.dram_tensor("bv", [GC], BF16, kind="ExternalInput").ap()
    out = nc.dram_tensor("out", [S, D], F32, kind="ExternalOutput").ap()

    with ExitStack() as ctx:
        tc = ctx.enter_context(TileContextCompat(nc))
        _emit(ctx, tc, xqt, xkt, xvt, wqt, wkt, wvt, wot, bq, bk, bv, out)

    fixed = _split_multi_waits(nc.to_json_bytes())
    nc.to_json_bytes = lambda: fixed
    return nc


def _split_multi_waits(raw):
    """Walrus here accepts only one sync-wait per instruction; hoist extras
    onto wait-only EventSemaphore instructions on the same engine."""
    m = json.loads(raw)
    counter = [0]

    def fix_block(b):
        new = []
        for inst in b.get("instructions", []):
            si = inst.get("sync_info")
            if si and si.get("on_wait") and len(si["on_wait"]) > 1:
                waits = si["on_wait"]
                for w in waits[:-1]:
                    counter[0] += 1
                    new.append({
                        "debug": inst.get("debug", 0),
                        "engine": inst["engine"],
                        "ins": [],
                        "outs": [],
                        "name": f"I-wsplit-{counter[0]}",
                        "opcode": "EventSemaphore",
                        "sync_info": {"on_update": [], "on_wait": [w]},
                    })
                si["on_wait"] = waits[-1:]
            new.append(inst)
        b["instructions"] = new
        for sub in b.get("blocks", []):
            fix_block(sub)

    for fn in m["functions"]:
        for b in fn.get("blocks", []):
            fix_block(b)
    return json.dumps(m).encode()


def _emit(ctx, tc, xqt, xkt, xvt, wqt, wkt, wvt, wot, bq, bk, bv, out):
    nc = tc.nc

    persist = ctx.enter_context(tc.tile_pool(name="persist", bufs=1))
    xs = ctx.enter_context(tc.tile_pool(name="xs", bufs=3))
    stp = ctx.enter_context(tc.tile_pool(name="st", bufs=2, space="PSUM"))
    opp = ctx.enter_context(tc.tile_pool(name="op", bufs=2, space="PSUM"))
    pap = ctx.enter_context(tc.tile_pool(name="pa", bufs=3))
    sap = ctx.enter_context(tc.tile_pool(name="sa", bufs=3))
    zp = ctx.enter_context(tc.tile_pool(name="zp", bufs=2))
    otp = ctx.enter_context(tc.tile_pool(name="ot", bufs=2))
    catp = ctx.enter_context(tc.tile_pool(name="cat", bufs=2))

    # ------------------------------------------------------- persistent
    qT = persist.tile([P, 2, S], F32R)
    kT = persist.tile([P, 2, S], F32R)
    vA = persist.tile([P, KVB, HG * (DK + 1)], F32R)
    wq_s = persist.tile([P, D // P, GC], BF16)
    wk_s = persist.tile([P, D // P, GC], BF16)
    wv_s = persist.tile([P, D // P, GC], BF16)
    wo_s = persist.tile([P, GC // P, D], F32R)
    bq_r = persist.tile([1, GC], BF16)
    bk_r = persist.tile([1, GC], BF16)
    bv_r = persist.tile([1, GC], BF16)

    # ones rows (memset + DVE self-copy so they count as f32r-produced)
    ones_row = persist.tile([1, QC], BF16)
    nc.vector.memset(ones_row, 1.0)
    ones_bcr = persist.tile([1, DK], BF16)
    nc.vector.memset(ones_bcr, 1.0)
    # V ones columns
    nc.vector.memset(vA.bitcast(F32), 1.0)
    ones_cols = vA[:].rearrange("p s (h c) -> p s h c", h=HG)[:, :, :, DK:]
    nc.vector.tensor_copy(ones_cols, ones_cols.bitcast(F32))

    # ------------------------------------------------------ weight DMAs
    # (K and Q weights first: the first ST pair needs only kT/qT chunk 0;
    # V weights and chunks follow just in time for the lagged PV matmuls.)
    def dma_w(w_dram, w_sb):
        nc.sync.dma_start(
            w_sb, w_dram.rearrange("(kc p) c -> p kc c", p=P))

    dma_w(wkt, wk_s)
    dma_w(wqt, wq_s)
    nc.gpsimd.dma_start(bq_r, bq[None, :])
    nc.gpsimd.dma_start(bk_r, bk[None, :])
    nc.gpsimd.dma_start(bv_r, bv[None, :])

    # PE warmup: a continuous chain of tiny matmuls spans the initial DMA
    # window so the PE p-state is fully ramped when the projections start.
    wup = stp.tile([P, 2 * QC], F32, tag="st", name="wup")
    for _ in range(250):
        nc.tensor.matmul(wup[0:1, 0:DK], ones_row[0:1, 0:1],
                         ones_row[0:1, 0:DK], start=True, stop=True)

    exit_ctr = [0]

    def score_exit(dst, src):
        exit_ctr[0] += 1
        if ACT_EXIT_EVERY and exit_ctr[0] % ACT_EXIT_EVERY == 0:
            nc.scalar.activation(dst, src, AF.Abs)
        else:
            du, su = dst.bitcast(mybir.dt.uint32), src.bitcast(mybir.dt.uint32)
            nc.vector.tensor_scalar(du, su, 0x7FFFFFFF, None, ALU.bitwise_and)

    def proj_dma(x_dram, sc):
        xt = xs.tile([P, D // P, SC], BF16, tag="xs")
        nc.sync.dma_start(
            xt, x_dram.rearrange("(kc p) s -> p kc s", p=P)
            [:, :, sc * SC:(sc + 1) * SC])
        return xt

    def proj_mm(xt, sc, which):
        """Matmuls+exit for one SC-chunk of a projection. which: 'k'|'v'|'q'."""
        ps = stp.tile([P, 2 * QC], F32, tag="st", name="pj")
        if which in ("k", "q"):
            w_sb, dstT, b_r = ((wk_s, kT, bk_r) if which == "k"
                               else (wq_s, qT, bq_r))
            for m in range(2):
                half = ps[:, m * SC:(m + 1) * SC]
                for kc in range(D // P):
                    nc.tensor.matmul(
                        half, w_sb[:, kc, m * P:(m + 1) * P],
                        xt[:, kc, :], start=(kc == 0), stop=False)
                nc.tensor.matmul(
                    half, b_r[0:1, m * P:(m + 1) * P],
                    ones_row, start=False, stop=True)
            dst = dstT[:, :, sc * SC:(sc + 1) * SC]
            nc.scalar.activation(
                dst, ps[:].rearrange("p (m f) -> p m f", m=2), AF.Copy)
        else:
            for sb in range(SC // P):
                seg = ps[:, sb * GC:(sb + 1) * GC]
                for kc in range(D // P):
                    nc.tensor.matmul(
                        seg, xt[:, kc, sb * P:(sb + 1) * P],
                        wv_s[:, kc, :], start=(kc == 0), stop=False)
                nc.tensor.matmul(seg, ones_row[0:1, 0:P],
                                 bv_r, start=False, stop=True)
            gsb = sc * (SC // P)
            dst = vA[:, gsb:gsb + 4, :].rearrange(
                "p s (h c) -> p s h c", h=HG)[:, :, :, :DK]
            src = ps[:].rearrange("p (s h c) -> p s h c", s=4, h=HG)
            nc.scalar.activation(dst, src, AF.Copy)

    def proj_chunk(x_dram, sc, which):
        proj_mm(proj_dma(x_dram, sc), sc, which)

    # K, Q chunk 0 first (unblocks the first ST pair), then V weights +
    # chunk 0 (needed one pair later by the lagged PVs), then Wo. The
    # remaining K/V chunks interleave into the first attention block.
    proj_chunk(xkt, 0, "k")
    proj_chunk(xqt, 0, "q")
    dma_w(wvt, wv_s)
    proj_chunk(xvt, 0, "v")

    # ---------------------------------------------------- attention
    # Two levels of software pipelining against the in-order PE queue:
    #  - PV matmuls run one kv-pair behind their STs (never wait on abs/exp)
    #  - each block's normalization/out-projection tail is deferred into the
    #    middle of the NEXT block, so block boundaries don't stall DVE/ACT.
    pending_pv = []
    pending_tail = []

    def flush_pv():
        for mm in pending_pv:
            mm()
        pending_pv.clear()

    def flush_tail(n=None):
        take = len(pending_tail) if n is None else min(n, len(pending_tail))
        for f in pending_tail[:take]:
            f()
        del pending_tail[:take]

    cat_tiles = {}

    def get_cat(qc):
        if qc not in cat_tiles:
            cat_tiles[qc] = catp.tile([P, 2, QC], F32R, tag="cat", name=f"cat{qc}")
        return cat_tiles[qc]

    def norm_a(qc, pr, opt):
        catT = get_cat(qc)
        # 1/Z via exp(-ln(Z)) on ACT: certified f32r writes, PSUM-direct read
        lnz = zp.tile([1, 2 * QC], F32, tag="z")
        nc.scalar.activation(
            lnz, opt[:].rearrange("p (h f) -> p h f", h=2)[64:65, :, :], AF.Ln)
        rec = zp.tile([1, 2 * QC], BF16, tag="rec")
        nc.scalar.activation(rec, lnz, AF.Exp, scale=-1.0)
        for half in range(2):
            csl = slice(half * DK, (half + 1) * DK)
            dst = catT[csl, pr, :]
            nc.scalar.activation(
                dst, opt[0:DK, half * QC:(half + 1) * QC], AF.Copy)
        return rec

    def norm_b(qc, pr, rec):
        catT = get_cat(qc)
        bc2 = opp.tile([P, 2 * QC], F32, tag="o", name="bc")
        for half in range(2):
            csl = slice(half * DK, (half + 1) * DK)
            bc = bc2[csl, half * QC:(half + 1) * QC]
            nc.tensor.matmul(bc, ones_bcr,
                             rec[0:1, half * QC:(half + 1) * QC],
                             start=True, stop=True,
                             tile_position=(0, half * DK))
            dst = catT[csl, pr, :]
            nc.vector.tensor_tensor(dst, dst, bc.bitcast(F32R), ALU.mult)

    def norm(qc, pr, opt):
        norm_b(qc, pr, norm_a(qc, pr, opt))

    def outproj(qc, js):
        catT = get_cat(qc)
        for j in js:
            sb = qc * (QC // P) + j
            o_t = otp.tile([P, D], F32, tag="ot")
            po = stp.tile([P, 2 * QC], F32, tag="st", name="po")
            for nn in range(2):
                seg = po[:, nn * QC:(nn + 1) * QC]
                for kc in range(2):
                    nc.tensor.matmul(
                        seg, catT[:, kc, j * P:(j + 1) * P],
                        wo_s[:, kc, nn * QC:(nn + 1) * QC],
                        start=(kc == 0), stop=(kc == 1))
            nc.scalar.activation(o_t, po, AF.Copy)
            nc.sync.dma_start(out[sb * P:(sb + 1) * P, :], o_t)

    def attn_pair(qc, pr, pv, opt):
        qsl = slice(qc * QC, (qc + 1) * QC)
        hA, hB = 2 * pr, 2 * pr + 1
        pa_t = pap.tile([P, 2 * 2 * QC], F32R, tag="pa")
        sabs = sap.tile([P, 2 * 2 * QC], F32, tag="sa")
        for i in range(2):
            kv = 2 * pv + i
            ksl = slice(kv * P, (kv + 1) * P)
            st = stp.tile([P, 2 * QC], F32, tag="st", name="stt")
            nc.tensor.matmul(
                st[:, :QC], kT[0:DK, pr, ksl],
                qT[0:DK, pr, qsl], start=True, stop=True,
                tile_position=(0, 0))
            nc.tensor.matmul(
                st[:, QC:], kT[DK:P, pr, ksl],
                qT[DK:P, pr, qsl], start=True, stop=True,
                tile_position=(DK, 0))
            score_exit(sabs[:, i * 2 * QC:(i + 1) * 2 * QC], st)
        flush_pv()
        nc.scalar.activation(pa_t, sabs, AF.Exp, scale=SCALE)

        def mk_pv():
            for i in range(2):
                kv = 2 * pv + i
                off = i * 2 * QC
                nc.tensor.matmul(
                    opt[0:DK + 1, :QC],
                    vA[:, kv, hA * 65:hA * 65 + 65],
                    pa_t[:, off:off + QC],
                    start=(kv == 0), stop=(kv == KVB - 1))
                nc.tensor.matmul(
                    opt[0:DK + 1, QC:],
                    vA[:, kv, hB * 65:hB * 65 + 65],
                    pa_t[:, off + QC:off + 2 * QC],
                    start=(kv == 0), stop=(kv == KVB - 1))
        pending_pv.append(mk_pv)

    # --- qc 0: pr0/pr1 pair streams merged so both head-pairs' abs/exp work
    # rides each arriving K/V chunk (DMA otherwise starves DVE/ACT here).
    opt0 = opp.tile([P, 2 * QC], F32, tag="o")
    opt1 = opp.tile([P, 2 * QC], F32, tag="o")
    kv_tiles = {}
    xq1_tile = None
    for pv in range(KVB // 2):
        # prefetch chunk DMAs one pair ahead of their matmuls (xs bufs=3)
        if pv in (0, 2, 4):
            sc = pv // 2 + 1
            kv_tiles[sc] = [proj_dma(xkt, sc)]
        if pv in (1, 3, 5):
            sc = (pv + 1) // 2
            kv_tiles[sc].append(proj_dma(xvt, sc))
        if pv in (2, 4, 6):
            sc = pv // 2
            flush_pv()
            xk_t, xv_t = kv_tiles.pop(sc)
            proj_mm(xk_t, sc, "k")
            proj_mm(xv_t, sc, "v")
        attn_pair(0, 0, pv, opt0)
        if pv == KVB // 2 - 1:
            flush_pv()
            norm(0, 0, opt0)
        attn_pair(0, 1, pv, opt1)
        if pv == 5:
            nc.sync.dma_start(
                wo_s, wot.bitcast(F32R).rearrange("(kc p) d -> p kc d", p=P))
        if pv == 6:
            xq1_tile = proj_dma(xqt, 1)
        if pv == 7:
            proj_mm(xq1_tile, 1, "q")
    flush_pv()
    pending_tail.append(lambda: norm(0, 1, opt1))
    pending_tail.append(lambda: outproj(0, (0, 1)))
    pending_tail.append(lambda: outproj(0, (2, 3)))

    # --- qc 1..3: per-(qc, pr) blocks with deferred tails
    for qc in range(1, S // QC):
        for pr in range(2):
            opt = opp.tile([P, 2 * QC], F32, tag="o")
            for pv in range(KVB // 2):
                attn_pair(qc, pr, pv, opt)
                if pv >= 1:
                    flush_tail(1)
            if pr == 0:
                box = {}

                def t_na(qc=qc, pr=pr, opt=opt, box=box):
                    box["rec"] = norm_a(qc, pr, opt)

                def t_nb(qc=qc, pr=pr, box=box):
                    norm_b(qc, pr, box["rec"])
                pending_tail.append(t_na)
                pending_tail.append(t_nb)
                if qc + 1 < S // QC:
                    pending_tail.append(
                        lambda qc=qc: proj_chunk(xqt, qc + 1, "q"))
            else:
                box = {}

                def t_na(qc=qc, pr=pr, opt=opt, box=box):
                    box["rec"] = norm_a(qc, pr, opt)

                def t_nb(qc=qc, pr=pr, box=box):
                    norm_b(qc, pr, box["rec"])
                pending_tail.append(t_na)
                pending_tail.append(t_nb)
                for j in range(4):
                    pending_tail.append(lambda qc=qc, j=j: outproj(qc, (j,)))
    flush_pv()
    flush_tail()


_NC_CACHE = {}


def _get_nc():
    if "nc" not in _NC_CACHE:
        _NC_CACHE["nc"] = build_nc()
    return _NC_CACHE["nc"]


def make_in_maps(q, k, v, Wq, bq, Wk, bk, Wv, bv, Wo, bo):
    import ml_dtypes
    bf16 = ml_dtypes.bfloat16
    xT = [np.ascontiguousarray(np.asarray(a, np.float32).T.astype(bf16))
          for a in (q[0], k[0], v[0], q[1], k[1], v[1])]
    in_maps = []
    for c in range(N_CORES):
        b, g = divmod(c, 4)
        sl = slice(g * GC, (g + 1) * GC)
        in_maps.append({
            "xqt": xT[3 * b + 0],
            "xkt": xT[3 * b + 1],
            "xvt": xT[3 * b + 2],
            "wqt": np.ascontiguousarray(Wq[sl].T.astype(bf16)),
            "wkt": np.ascontiguousarray(Wk[sl].T.astype(bf16)),
            "wvt": np.ascontiguousarray(Wv[sl].T.astype(bf16)),
            "wot": np.ascontiguousarray(Wo[:, sl].T),
            "bq": np.ascontiguousarray(bq[sl].astype(bf16)),
            "bk": np.ascontiguousarray(bk[sl].astype(bf16)),
            "bv": np.ascontiguousarray(bv[sl].astype(bf16)),
        })
    return in_maps


def kernel(q, k, v, Wq, bq, Wk, bk, Wv, bv, Wo, bo, _trace=False):
    from concourse.bass_utils import run_bass_kernel_spmd

    q, k, v = (np.asarray(a, np.float32) for a in (q, k, v))
    Wq, bq, Wk, bk, Wv, bv, Wo, bo = (
        np.asarray(a, np.float32) for a in (Wq, bq, Wk, bk, Wv, bv, Wo, bo))

    nc = _get_nc()
    in_maps = make_in_maps(q, k, v, Wq, bq, Wk, bk, Wv, bv, Wo, bo)
    res = run_bass_kernel_spmd(nc, in_maps, core_ids=list(range(N_CORES)),
                               trace=_trace)
    partials = np.stack([r["out"] for r in res.results])  # [8, S, D]
    full = partials.reshape(2, 4, S, D).sum(axis=1) + bo[None, None, :]
    if _trace:
        return full.astype(np.float32), res
    return full.astype(np.float32)


# revision 4
# speedup vs baseline: 1.0462x; 1.0235x over previous
"""Trainium2 Bass kernel for nn_MultiHeadAttention_70866960384614.

MHA: B=2, S=2048, D_MODEL=1024, HEADS=16, D_K=64, softmax(|QK^T|/8) @ V.

Sharding (8 cores): data-parallel over batch (2) x tensor-parallel over head
groups (4 groups of 4 heads). Host pre-transposes x and the weight slices so
the device does ZERO layout transposes: per core the inputs are
  xqT/xkT/xvT [1024, 2048]  (x^T, fp32)
  wqT/wkT/wvT [1024, 256]   (W[group].T)
  woT         [256, 1024]   (Wo[:, group].T)
DMA feeds matmul operands directly as float32r (full-rate fp32 on the PE).

Per-core dataflow:
  QT = Wq @ xT (+bq)            [256, 2048]   (= wqT.T @ xqT)
  KT likewise; V = x @ WvT + bv  stored kv-major with a ones column per head
  per (qchunk, head-pair): for each kv pair:
     ST = K Q^T (2 heads packed in the PE array), DVE/ACT |.| exit to SBUF,
     one bulk ACT exp over [128, 2048], PV accumulates [V|1].T @ P
  normalization: Z rows -> DVE reciprocal -> PE broadcast -> DVE multiply
  out partial = catT.T @ WoT -> DMA (host sums the 4 group partials + bo)
"""

import json
from contextlib import ExitStack

import numpy as np

import concourse.bass as bass
import concourse.mybir as mybir
import concourse.tile as tile
from concourse.vector_clock import ScopedClock

F32 = mybir.dt.float32
F32R = mybir.dt.float32r
BF16 = mybir.dt.bfloat16
AF = mybir.ActivationFunctionType
ALU = mybir.AluOpType

S = 2048
D = 1024
HG = 4            # heads per core
DK = 64
GC = HG * DK      # 256
P = 128
N_CORES = 8
SC = 512          # s-chunk for projection streaming
QC = 512          # q-chunk within attention
KVB = S // P      # 16 kv blocks
SCALE = 0.125

# every ACT_EXIT_EVERYth score-exit goes to ACT instead of DVE (engine balance)
ACT_EXIT_EVERY = 0   # 0 = never


class TileContextCompat(tile.TileContext):
    """This container's walrus build rejects >1 sync-wait on a CTRL (Drain)
    instruction; spread the kernel-tail DMA-lane waits across one drain
    each instead of piling them on a single drain."""

    def _drain_and_barrier(self, tick_clock, wait_clock):
        drain_inst = self.nc.sync.drain()
        wait_clock.add_sem_waits(
            drain_inst.ins, ScopedClock({None: tick_clock.global_clock}))
        si = drain_inst.ins.sync_info
        extra = []
        if si is not None and si.on_wait is not None:
            while len(si.on_wait) > 1:
                extra.append(si.on_wait.pop())
        for w in extra:
            d2 = self.nc.sync.drain()
            if d2.ins.sync_info is None:
                d2.ins.sync_info = mybir.SyncInfo(on_wait=[w], on_update=[])
            else:
                d2.ins.sync_info.on_wait.append(w)
        self.nc.all_engine_barrier()
        assert self.sems is not None
        popped = self.nc._tile_sem_poison_stack.pop()
        assert popped is self._sem_poison
        self.nc.clear_and_free_semaphores(list(self.sems.allocated().values()))
        self.nc.all_engine_barrier()


def build_nc():
    nc = bass.Bass("TRN2", target_bir_lowering=False, debug=False,
                   num_devices=N_CORES)

    xqt = nc.dram_tensor("xqt", [D, S], BF16, kind="ExternalInput").ap()
    xkt = nc.dram_tensor("xkt", [D, S], BF16, kind="ExternalInput").ap()
    xvt = nc.dram_tensor("xvt", [D, S], BF16, kind="ExternalInput").ap()
    wqt = nc.dram_tensor("wqt", [D, GC], BF16, kind="ExternalInput").ap()
    wkt = nc.dram_tensor("wkt", [D, GC], BF16, kind="ExternalInput").ap()
    wvt = nc.dram_tensor("wvt", [D, GC], BF16, kind="ExternalInput").ap()
    wot = nc.dram_tensor("wot", [GC, D], F32, kind="ExternalInput").ap()
    bq = nc.dram_tensor("bq", [GC], BF16, kind="ExternalInput").ap()
    bk = nc.dram_tensor("bk", [GC], BF16, kind="ExternalInput").ap()
    bv = nc.dram_tensor("bv", [GC], BF16, kind="ExternalInput").ap()
    out = nc.dram_tensor("out", [S, D], F32, kind="ExternalOutput").ap()

    with ExitStack() as ctx:
        tc = ctx.enter_context(TileContextCompat(nc))
        _emit(ctx, tc, xqt, xkt, xvt, wqt, wkt, wvt, wot, bq, bk, bv, out)

    fixed = _split_multi_waits(nc.to_json_bytes())
    nc.to_json_bytes = lambda: fixed
    return nc


def _split_multi_waits(raw):
    """Walrus here accepts only one sync-wait per instruction; hoist extras
    onto wait-only EventSemaphore instructions on the same engine."""
    m = json.loads(raw)
    counter = [0]

    def fix_block(b):
        new = []
        for inst in b.get("instructions", []):
            si = inst.get("sync_info")
            if si and si.get("on_wait") and len(si["on_wait"]) > 1:
                waits = si["on_wait"]
                for w in waits[:-1]:
                    counter[0] += 1
                    new.append({
                        "debug": inst.get("debug", 0),
                        "engine": inst["engine"],
                        "ins": [],
                        "outs": [],
                        "name": f"I-wsplit-{counter[0]}",
                        "opcode": "EventSemaphore",
                        "sync_info": {"on_update": [], "on_wait": [w]},
                    })
                si["on_wait"] = waits[-1:]
            new.append(inst)
        b["instructions"] = new
        for sub in b.get("blocks", []):
            fix_block(sub)

    for fn in m["functions"]:
        for b in fn.get("blocks", []):
            fix_block(b)
    return json.dumps(m).encode()


def _emit(ctx, tc, xqt, xkt, xvt, wqt, wkt, wvt, wot, bq, bk, bv, out):
    nc = tc.nc

    persist = ctx.enter_context(tc.tile_pool(name="persist", bufs=1))
    xs = ctx.enter_context(tc.tile_pool(name="xs", bufs=3))
    stp = ctx.enter_context(tc.tile_pool(name="st", bufs=2, space="PSUM"))
    opp = ctx.enter_context(tc.tile_pool(name="op", bufs=2, space="PSUM"))
    pap = ctx.enter_context(tc.tile_pool(name="pa", bufs=3))
    sap = ctx.enter_context(tc.tile_pool(name="sa", bufs=3))
    zp = ctx.enter_context(tc.tile_pool(name="zp", bufs=2))
    otp = ctx.enter_context(tc.tile_pool(name="ot", bufs=4))
    catp = ctx.enter_context(tc.tile_pool(name="cat", bufs=2))

    # ------------------------------------------------------- persistent
    qT = persist.tile([P, 2, S], F32R)
    kT = persist.tile([P, 2, S], F32R)
    vA = persist.tile([P, KVB, HG * (DK + 1)], F32R)
    wq_s = persist.tile([P, D // P, GC], BF16)
    wk_s = persist.tile([P, D // P, GC], BF16)
    wv_s = persist.tile([P, D // P, GC], BF16)
    wo_s = persist.tile([P, GC // P, D], F32R)
    bq_r = persist.tile([1, GC], BF16)
    bk_r = persist.tile([1, GC], BF16)
    bv_r = persist.tile([1, GC], BF16)

    # ones rows (memset + DVE self-copy so they count as f32r-produced)
    ones_row = persist.tile([1, QC], BF16)
    nc.vector.memset(ones_row, 1.0)
    ones_bcr = persist.tile([1, DK], BF16)
    nc.vector.memset(ones_bcr, 1.0)
    # V ones columns
    nc.vector.memset(vA.bitcast(F32), 1.0)
    ones_cols = vA[:].rearrange("p s (h c) -> p s h c", h=HG)[:, :, :, DK:]
    nc.vector.tensor_copy(ones_cols, ones_cols.bitcast(F32))

    # ------------------------------------------------------ weight DMAs
    # (K and Q weights first: the first ST pair needs only kT/qT chunk 0;
    # V weights and chunks follow just in time for the lagged PV matmuls.)
    def dma_w(w_dram, w_sb):
        nc.sync.dma_start(
            w_sb, w_dram.rearrange("(kc p) c -> p kc c", p=P))

    dma_w(wkt, wk_s)
    dma_w(wqt, wq_s)
    nc.gpsimd.dma_start(bq_r, bq[None, :])
    nc.gpsimd.dma_start(bk_r, bk[None, :])
    nc.gpsimd.dma_start(bv_r, bv[None, :])

    # PE warmup: a continuous chain of tiny matmuls spans the initial DMA
    # window so the PE p-state is fully ramped when the projections start.
    wup = stp.tile([P, 2 * QC], F32, tag="st", name="wup")
    for _ in range(150):
        nc.tensor.matmul(wup[0:1, 0:DK], ones_row[0:1, 0:1],
                         ones_row[0:1, 0:DK], start=True, stop=True)

    exit_ctr = [0]

    def score_exit(dst, src):
        exit_ctr[0] += 1
        if ACT_EXIT_EVERY and exit_ctr[0] % ACT_EXIT_EVERY == 0:
            nc.scalar.activation(dst, src, AF.Abs)
        else:
            du, su = dst.bitcast(mybir.dt.uint32), src.bitcast(mybir.dt.uint32)
            nc.vector.tensor_scalar(du, su, 0x7FFFFFFF, None, ALU.bitwise_and)

    def proj_dma(x_dram, sc, split=False):
        xt = xs.tile([P, D // P, SC], BF16, tag="xs")
        src = x_dram.rearrange("(kc p) s -> p kc s", p=P)[
            :, :, sc * SC:(sc + 1) * SC]
        if split:
            # per-kc DMAs: accumulation matmuls start as each slice lands
            for kc in range(D // P):
                nc.sync.dma_start(xt[:, kc:kc + 1, :], src[:, kc:kc + 1, :])
        else:
            nc.sync.dma_start(xt, src)
        return xt

    def proj_mm(xt, sc, which):
        """Matmuls+exit for one SC-chunk of a projection. which: 'k'|'v'|'q'."""
        ps = stp.tile([P, 2 * QC], F32, tag="st", name="pj")
        if which in ("k", "q"):
            w_sb, dstT, b_r = ((wk_s, kT, bk_r) if which == "k"
                               else (wq_s, qT, bq_r))
            for m in range(2):
                half = ps[:, m * SC:(m + 1) * SC]
                for kc in range(D // P):
                    nc.tensor.matmul(
                        half, w_sb[:, kc, m * P:(m + 1) * P],
                        xt[:, kc, :], start=(kc == 0), stop=False)
                nc.tensor.matmul(
                    half, b_r[0:1, m * P:(m + 1) * P],
                    ones_row, start=False, stop=True)
            dst = dstT[:, :, sc * SC:(sc + 1) * SC]
            nc.scalar.activation(
                dst, ps[:].rearrange("p (m f) -> p m f", m=2), AF.Copy)
        else:
            for sb in range(SC // P):
                seg = ps[:, sb * GC:(sb + 1) * GC]
                for kc in range(D // P):
                    nc.tensor.matmul(
                        seg, xt[:, kc, sb * P:(sb + 1) * P],
                        wv_s[:, kc, :], start=(kc == 0), stop=False)
                nc.tensor.matmul(seg, ones_row[0:1, 0:P],
                                 bv_r, start=False, stop=True)
            gsb = sc * (SC // P)
            dstv = vA[:, gsb:gsb + 4, :].rearrange(
                "p s (h c) -> p s h c", h=HG)[:, :, :, :DK]
            srcv = ps[:].rearrange("p (s h c) -> p s h c", s=4, h=HG)
            nc.scalar.activation(dstv, srcv, AF.Copy)

    def proj_chunk(x_dram, sc, which):
        proj_mm(proj_dma(x_dram, sc), sc, which)

    # K, Q chunk 0 first (unblocks the first ST pair), then V weights +
    # chunk 0 (needed one pair later by the lagged PVs), then Wo. The
    # remaining K/V chunks interleave into the first attention block.
    proj_chunk(xkt, 0, "k")
    proj_chunk(xqt, 0, "q")
    dma_w(wvt, wv_s)
    proj_chunk(xvt, 0, "v")

    # ---------------------------------------------------- attention
    # Two levels of software pipelining against the in-order PE queue:
    #  - PV matmuls run one kv-pair behind their STs (never wait on abs/exp)
    #  - each block's normalization/out-projection tail is deferred into the
    #    middle of the NEXT block, so block boundaries don't stall DVE/ACT.
    pending_pv = []
    pending_tail = []

    def flush_pv():
        for mm in pending_pv:
            mm()
        pending_pv.clear()

    def flush_tail(n=None):
        take = len(pending_tail) if n is None else min(n, len(pending_tail))
        for f in pending_tail[:take]:
            f()
        del pending_tail[:take]

    cat_tiles = {}

    def get_cat(qc):
        if qc not in cat_tiles:
            cat_tiles[qc] = catp.tile([P, 2, QC], F32R, tag="cat", name=f"cat{qc}")
        return cat_tiles[qc]

    def norm_a(qc, pr, opt):
        catT = get_cat(qc)
        # 1/Z via exp(-ln(Z)) on ACT: certified f32r writes, PSUM-direct read
        lnz = zp.tile([1, 2 * QC], F32, tag="z")
        nc.scalar.activation(
            lnz, opt[:].rearrange("p (h f) -> p h f", h=2)[64:65, :, :], AF.Ln)
        rec = zp.tile([1, 2 * QC], BF16, tag="rec")
        nc.scalar.activation(rec, lnz, AF.Exp, scale=-1.0)
        for half in range(2):
            csl = slice(half * DK, (half + 1) * DK)
            dst = catT[csl, pr, :]
            nc.scalar.activation(
                dst, opt[0:DK, half * QC:(half + 1) * QC], AF.Copy)
        return rec

    def norm_b(qc, pr, rec):
        catT = get_cat(qc)
        bc2 = opp.tile([P, 2 * QC], F32, tag="o", name="bc")
        for half in range(2):
            csl = slice(half * DK, (half + 1) * DK)
            bc = bc2[csl, half * QC:(half + 1) * QC]
            nc.tensor.matmul(bc, ones_bcr,
                             rec[0:1, half * QC:(half + 1) * QC],
                             start=True, stop=True,
                             tile_position=(0, half * DK))
            dst = catT[csl, pr, :]
            nc.vector.tensor_tensor(dst, dst, bc.bitcast(F32R), ALU.mult)

    def norm(qc, pr, opt):
        norm_b(qc, pr, norm_a(qc, pr, opt))

    def outproj(qc, js):
        catT = get_cat(qc)
        for j in js:
            sb = qc * (QC // P) + j
            o_t = otp.tile([P, D], F32, tag="ot")
            po = stp.tile([P, 2 * QC], F32, tag="st", name="po")
            for nn in range(2):
                seg = po[:, nn * QC:(nn + 1) * QC]
                for kc in range(2):
                    nc.tensor.matmul(
                        seg, catT[:, kc, j * P:(j + 1) * P],
                        wo_s[:, kc, nn * QC:(nn + 1) * QC],
                        start=(kc == 0), stop=(kc == 1))
            nc.scalar.activation(o_t, po, AF.Copy)
            nc.sync.dma_start(out[sb * P:(sb + 1) * P, :], o_t)

    def attn_pair(qc, pr, pv, opt):
        qsl = slice(qc * QC, (qc + 1) * QC)
        hA, hB = 2 * pr, 2 * pr + 1
        pa_t = pap.tile([P, 2 * 2 * QC], F32R, tag="pa")
        sabs = sap.tile([P, 2 * 2 * QC], F32, tag="sa")
        for i in range(2):
            kv = 2 * pv + i
            ksl = slice(kv * P, (kv + 1) * P)
            st = stp.tile([P, 2 * QC], F32, tag="st", name="stt")
            nc.tensor.matmul(
                st[:, :QC], kT[0:DK, pr, ksl],
                qT[0:DK, pr, qsl], start=True, stop=True,
                tile_position=(0, 0))
            nc.tensor.matmul(
                st[:, QC:], kT[DK:P, pr, ksl],
                qT[DK:P, pr, qsl], start=True, stop=True,
                tile_position=(DK, 0))
            score_exit(sabs[:, i * 2 * QC:(i + 1) * 2 * QC], st)
        flush_pv()
        nc.scalar.activation(pa_t, sabs, AF.Exp, scale=SCALE)

        def mk_pv():
            for i in range(2):
                kv = 2 * pv + i
                off = i * 2 * QC
                nc.tensor.matmul(
                    opt[0:DK + 1, :QC],
                    vA[:, kv, hA * 65:hA * 65 + 65],
                    pa_t[:, off:off + QC],
                    start=(kv == 0), stop=(kv == KVB - 1))
                nc.tensor.matmul(
                    opt[0:DK + 1, QC:],
                    vA[:, kv, hB * 65:hB * 65 + 65],
                    pa_t[:, off + QC:off + 2 * QC],
                    start=(kv == 0), stop=(kv == KVB - 1))
        pending_pv.append(mk_pv)

    # --- qc 0: pr0/pr1 pair streams merged so both head-pairs' abs/exp work
    # rides each arriving K/V chunk (DMA otherwise starves DVE/ACT here).
    opt0 = opp.tile([P, 2 * QC], F32, tag="o")
    opt1 = opp.tile([P, 2 * QC], F32, tag="o")
    kv_tiles = {}
    xq1_tile = None
    for pv in range(KVB // 2):
        # prefetch chunk DMAs one pair ahead of their matmuls (xs bufs=3)
        if pv in (0, 2, 4):
            sc = pv // 2 + 1
            kv_tiles[sc] = [proj_dma(xkt, sc)]
        if pv in (1, 3, 5):
            sc = (pv + 1) // 2
            kv_tiles[sc].append(proj_dma(xvt, sc))
        if pv in (2, 4, 6):
            sc = pv // 2
            flush_pv()
            xk_t, xv_t = kv_tiles.pop(sc)
            proj_mm(xk_t, sc, "k")
            proj_mm(xv_t, sc, "v")
        attn_pair(0, 0, pv, opt0)
        if pv == KVB // 2 - 1:
            flush_pv()
            norm(0, 0, opt0)
        attn_pair(0, 1, pv, opt1)
        if pv == 5:
            nc.sync.dma_start(
                wo_s, wot.bitcast(F32R).rearrange("(kc p) d -> p kc d", p=P))
        if pv == 6:
            xq1_tile = proj_dma(xqt, 1)
        if pv == 7:
            proj_mm(xq1_tile, 1, "q")
    flush_pv()
    pending_tail.append(lambda: norm(0, 1, opt1))
    pending_tail.append(lambda: outproj(0, (0, 1)))
    pending_tail.append(lambda: outproj(0, (2, 3)))

    # --- qc 1..3: per-(qc, pr) blocks with deferred tails
    for qc in range(1, S // QC):
        for pr in range(2):
            opt = opp.tile([P, 2 * QC], F32, tag="o")
            for pv in range(KVB // 2):
                attn_pair(qc, pr, pv, opt)
                if pv >= 1:
                    flush_tail(1)
            if pr == 0:
                box = {}

                def t_na(qc=qc, pr=pr, opt=opt, box=box):
                    box["rec"] = norm_a(qc, pr, opt)

                def t_nb(qc=qc, pr=pr, box=box):
                    norm_b(qc, pr, box["rec"])
                pending_tail.append(t_na)
                pending_tail.append(t_nb)
                if qc + 1 < S // QC:
                    pending_tail.append(
                        lambda qc=qc: proj_chunk(xqt, qc + 1, "q"))
            else:
                box = {}

                def t_na(qc=qc, pr=pr, opt=opt, box=box):
                    box["rec"] = norm_a(qc, pr, opt)

                def t_nb(qc=qc, pr=pr, box=box):
                    norm_b(qc, pr, box["rec"])
                pending_tail.append(t_na)
                pending_tail.append(t_nb)
                for j in range(4):
                    pending_tail.append(lambda qc=qc, j=j: outproj(qc, (j,)))
    flush_pv()
    flush_tail()


_NC_CACHE = {}


def _get_nc():
    if "nc" not in _NC_CACHE:
        _NC_CACHE["nc"] = build_nc()
    return _NC_CACHE["nc"]


def make_in_maps(q, k, v, Wq, bq, Wk, bk, Wv, bv, Wo, bo):
    import ml_dtypes
    bf16 = ml_dtypes.bfloat16
    xT = [np.ascontiguousarray(np.asarray(a, np.float32).T.astype(bf16))
          for a in (q[0], k[0], v[0], q[1], k[1], v[1])]
    in_maps = []
    for c in range(N_CORES):
        b, g = divmod(c, 4)
        sl = slice(g * GC, (g + 1) * GC)
        in_maps.append({
            "xqt": xT[3 * b + 0],
            "xkt": xT[3 * b + 1],
            "xvt": xT[3 * b + 2],
            "wqt": np.ascontiguousarray(Wq[sl].T.astype(bf16)),
            "wkt": np.ascontiguousarray(Wk[sl].T.astype(bf16)),
            "wvt": np.ascontiguousarray(Wv[sl].T.astype(bf16)),
            "wot": np.ascontiguousarray(Wo[:, sl].T),
            "bq": np.ascontiguousarray(bq[sl].astype(bf16)),
            "bk": np.ascontiguousarray(bk[sl].astype(bf16)),
            "bv": np.ascontiguousarray(bv[sl].astype(bf16)),
        })
    return in_maps


def kernel(q, k, v, Wq, bq, Wk, bk, Wv, bv, Wo, bo, _trace=False):
    from concourse.bass_utils import run_bass_kernel_spmd

    q, k, v = (np.asarray(a, np.float32) for a in (q, k, v))
    Wq, bq, Wk, bk, Wv, bv, Wo, bo = (
        np.asarray(a, np.float32) for a in (Wq, bq, Wk, bk, Wv, bv, Wo, bo))

    nc = _get_nc()
    in_maps = make_in_maps(q, k, v, Wq, bq, Wk, bk, Wv, bv, Wo, bo)
    res = run_bass_kernel_spmd(nc, in_maps, core_ids=list(range(N_CORES)),
                               trace=_trace)
    partials = np.stack([r["out"] for r in res.results])  # [8, S, D]
    full = partials.reshape(2, 4, S, D).sum(axis=1) + bo[None, None, :]
    if _trace:
        return full.astype(np.float32), res
    return full.astype(np.float32)


# revision 5
# speedup vs baseline: 1.0736x; 1.0262x over previous
"""Trainium2 Bass kernel for nn_MultiHeadAttention_70866960384614.

MHA: B=2, S=2048, D_MODEL=1024, HEADS=16, D_K=64, softmax(|QK^T|/8) @ V.

Sharding (8 cores): data-parallel over batch (2) x tensor-parallel over head
groups (4 groups of 4 heads). Host pre-transposes x and the weight slices so
the device does ZERO layout transposes: per core the inputs are
  xqT/xkT/xvT [1024, 2048]  (x^T, fp32)
  wqT/wkT/wvT [1024, 256]   (W[group].T)
  woT         [256, 1024]   (Wo[:, group].T)
DMA feeds matmul operands directly as float32r (full-rate fp32 on the PE).

Per-core dataflow:
  QT = Wq @ xT (+bq)            [256, 2048]   (= wqT.T @ xqT)
  KT likewise; V = x @ WvT + bv  stored kv-major with a ones column per head
  per (qchunk, head-pair): for each kv pair:
     ST = K Q^T (2 heads packed in the PE array), DVE/ACT |.| exit to SBUF,
     one bulk ACT exp over [128, 2048], PV accumulates [V|1].T @ P
  normalization: Z rows -> DVE reciprocal -> PE broadcast -> DVE multiply
  out partial = catT.T @ WoT -> DMA (host sums the 4 group partials + bo)
"""

import json
from contextlib import ExitStack

import numpy as np

import concourse.bass as bass
import concourse.mybir as mybir
import concourse.tile as tile
from concourse.vector_clock import ScopedClock

F32 = mybir.dt.float32
F32R = mybir.dt.float32r
BF16 = mybir.dt.bfloat16
AF = mybir.ActivationFunctionType
ALU = mybir.AluOpType

S = 2048
D = 1024
HG = 4            # heads per core
DK = 64
GC = HG * DK      # 256
P = 128
N_CORES = 8
SC = 512          # s-chunk for projection streaming
QC = 512          # q-chunk within attention
KVB = S // P      # 16 kv blocks
SCALE = 0.125

# every ACT_EXIT_EVERYth score-exit goes to ACT instead of DVE (engine balance)
ACT_EXIT_EVERY = 0   # 0 = never


class TileContextCompat(tile.TileContext):
    """This container's walrus build rejects >1 sync-wait on a CTRL (Drain)
    instruction; spread the kernel-tail DMA-lane waits across one drain
    each instead of piling them on a single drain."""

    def _drain_and_barrier(self, tick_clock, wait_clock):
        drain_inst = self.nc.sync.drain()
        wait_clock.add_sem_waits(
            drain_inst.ins, ScopedClock({None: tick_clock.global_clock}))
        si = drain_inst.ins.sync_info
        extra = []
        if si is not None and si.on_wait is not None:
            while len(si.on_wait) > 1:
                extra.append(si.on_wait.pop())
        for w in extra:
            d2 = self.nc.sync.drain()
            if d2.ins.sync_info is None:
                d2.ins.sync_info = mybir.SyncInfo(on_wait=[w], on_update=[])
            else:
                d2.ins.sync_info.on_wait.append(w)
        self.nc.all_engine_barrier()
        assert self.sems is not None
        popped = self.nc._tile_sem_poison_stack.pop()
        assert popped is self._sem_poison
        self.nc.clear_and_free_semaphores(list(self.sems.allocated().values()))
        self.nc.all_engine_barrier()


def build_nc():
    nc = bass.Bass("TRN2", target_bir_lowering=False, debug=False,
                   num_devices=N_CORES)

    xqt = nc.dram_tensor("xqt", [D, S], BF16, kind="ExternalInput").ap()
    xkt = nc.dram_tensor("xkt", [D, S], BF16, kind="ExternalInput").ap()
    xvt = nc.dram_tensor("xvt", [D, S], BF16, kind="ExternalInput").ap()
    wqt = nc.dram_tensor("wqt", [D, GC], BF16, kind="ExternalInput").ap()
    wkt = nc.dram_tensor("wkt", [D, GC], BF16, kind="ExternalInput").ap()
    wvt = nc.dram_tensor("wvt", [D, GC], BF16, kind="ExternalInput").ap()
    wot = nc.dram_tensor("wot", [GC, D], F32, kind="ExternalInput").ap()
    bq = nc.dram_tensor("bq", [GC], BF16, kind="ExternalInput").ap()
    bk = nc.dram_tensor("bk", [GC], BF16, kind="ExternalInput").ap()
    bv = nc.dram_tensor("bv", [GC], BF16, kind="ExternalInput").ap()
    out = nc.dram_tensor("out", [S, D], F32, kind="ExternalOutput").ap()

    with ExitStack() as ctx:
        tc = ctx.enter_context(TileContextCompat(nc))
        _emit(ctx, tc, xqt, xkt, xvt, wqt, wkt, wvt, wot, bq, bk, bv, out)

    fixed = _split_multi_waits(nc.to_json_bytes())
    nc.to_json_bytes = lambda: fixed
    return nc


def _split_multi_waits(raw):
    """Walrus here accepts only one sync-wait per instruction; hoist extras
    onto wait-only EventSemaphore instructions on the same engine."""
    m = json.loads(raw)
    counter = [0]

    def fix_block(b):
        new = []
        for inst in b.get("instructions", []):
            si = inst.get("sync_info")
            if si and si.get("on_wait") and len(si["on_wait"]) > 1:
                waits = si["on_wait"]
                for w in waits[:-1]:
                    counter[0] += 1
                    new.append({
                        "debug": inst.get("debug", 0),
                        "engine": inst["engine"],
                        "ins": [],
                        "outs": [],
                        "name": f"I-wsplit-{counter[0]}",
                        "opcode": "EventSemaphore",
                        "sync_info": {"on_update": [], "on_wait": [w]},
                    })
                si["on_wait"] = waits[-1:]
            new.append(inst)
        b["instructions"] = new
        for sub in b.get("blocks", []):
            fix_block(sub)

    for fn in m["functions"]:
        for b in fn.get("blocks", []):
            fix_block(b)
    return json.dumps(m).encode()


def _emit(ctx, tc, xqt, xkt, xvt, wqt, wkt, wvt, wot, bq, bk, bv, out):
    nc = tc.nc

    persist = ctx.enter_context(tc.tile_pool(name="persist", bufs=1))
    xs = ctx.enter_context(tc.tile_pool(name="xs", bufs=3))
    stp = ctx.enter_context(tc.tile_pool(name="st", bufs=2, space="PSUM"))
    opp = ctx.enter_context(tc.tile_pool(name="op", bufs=2, space="PSUM"))
    pap = ctx.enter_context(tc.tile_pool(name="pa", bufs=3))
    sap = ctx.enter_context(tc.tile_pool(name="sa", bufs=3))
    zp = ctx.enter_context(tc.tile_pool(name="zp", bufs=2))
    otp = ctx.enter_context(tc.tile_pool(name="ot", bufs=4))
    catp = ctx.enter_context(tc.tile_pool(name="cat", bufs=2))

    # ------------------------------------------------------- persistent
    qT = persist.tile([P, 2, S], F32R)
    kT = persist.tile([P, 2, S], F32R)
    vA = persist.tile([P, KVB, HG * (DK + 1)], F32R)
    wq_s = persist.tile([P, D // P, GC], BF16)
    wk_s = persist.tile([P, D // P, GC], BF16)
    wv_s = persist.tile([P, D // P, GC], BF16)
    wo_s = persist.tile([P, GC // P, D], F32R)
    bq_r = persist.tile([1, GC], BF16)
    bk_r = persist.tile([1, GC], BF16)
    bv_r = persist.tile([1, GC], BF16)

    # ones rows (memset + DVE self-copy so they count as f32r-produced)
    ones_row = persist.tile([1, QC], BF16)
    nc.vector.memset(ones_row, 1.0)
    ones_bcr = persist.tile([1, DK], BF16)
    nc.vector.memset(ones_bcr, 1.0)
    # V ones columns
    nc.vector.memset(vA.bitcast(F32), 1.0)
    ones_cols = vA[:].rearrange("p s (h c) -> p s h c", h=HG)[:, :, :, DK:]
    nc.vector.tensor_copy(ones_cols, ones_cols.bitcast(F32))

    # ------------------------------------------------------ weight DMAs
    # (K and Q weights first: the first ST pair needs only kT/qT chunk 0;
    # V weights and chunks follow just in time for the lagged PV matmuls.)
    def dma_w(w_dram, w_sb):
        nc.sync.dma_start(
            w_sb, w_dram.rearrange("(kc p) c -> p kc c", p=P))

    dma_w(wkt, wk_s)
    dma_w(wqt, wq_s)
    nc.gpsimd.dma_start(bq_r, bq[None, :])
    nc.gpsimd.dma_start(bk_r, bk[None, :])
    nc.gpsimd.dma_start(bv_r, bv[None, :])

    # PE warmup: a continuous chain of tiny matmuls spans the initial DMA
    # window so the PE p-state is fully ramped when the projections start.
    wup = stp.tile([P, 2 * QC], F32, tag="st", name="wup")
    for _ in range(150):
        nc.tensor.matmul(wup[0:1, 0:DK], ones_row[0:1, 0:1],
                         ones_row[0:1, 0:DK], start=True, stop=True)

    exit_ctr = [0]

    def score_exit(dst, src):
        exit_ctr[0] += 1
        if ACT_EXIT_EVERY and exit_ctr[0] % ACT_EXIT_EVERY == 0:
            nc.scalar.activation(dst, src, AF.Abs)
        else:
            du, su = dst.bitcast(mybir.dt.uint32), src.bitcast(mybir.dt.uint32)
            nc.vector.tensor_scalar(du, su, 0x7FFFFFFF, None, ALU.bitwise_and)

    def proj_dma(x_dram, sc, split=False):
        xt = xs.tile([P, D // P, SC], BF16, tag="xs")
        src = x_dram.rearrange("(kc p) s -> p kc s", p=P)[
            :, :, sc * SC:(sc + 1) * SC]
        if split:
            # per-kc DMAs: accumulation matmuls start as each slice lands
            for kc in range(D // P):
                nc.sync.dma_start(xt[:, kc:kc + 1, :], src[:, kc:kc + 1, :])
        else:
            nc.sync.dma_start(xt, src)
        return xt

    def proj_mm(xt, sc, which):
        """Matmuls+exit for one SC-chunk of a projection. which: 'k'|'v'|'q'."""
        ps = stp.tile([P, 2 * QC], F32, tag="st", name="pj")
        if which in ("k", "q"):
            w_sb, dstT, b_r = ((wk_s, kT, bk_r) if which == "k"
                               else (wq_s, qT, bq_r))
            for m in range(2):
                half = ps[:, m * SC:(m + 1) * SC]
                for kc in range(D // P):
                    nc.tensor.matmul(
                        half, w_sb[:, kc, m * P:(m + 1) * P],
                        xt[:, kc, :], start=(kc == 0), stop=False)
                nc.tensor.matmul(
                    half, b_r[0:1, m * P:(m + 1) * P],
                    ones_row, start=False, stop=True)
            dst = dstT[:, :, sc * SC:(sc + 1) * SC]
            nc.scalar.activation(
                dst, ps[:].rearrange("p (m f) -> p m f", m=2), AF.Copy)
        else:
            for sb in range(SC // P):
                seg = ps[:, sb * GC:(sb + 1) * GC]
                for kc in range(D // P):
                    nc.tensor.matmul(
                        seg, xt[:, kc, sb * P:(sb + 1) * P],
                        wv_s[:, kc, :], start=(kc == 0), stop=False)
                nc.tensor.matmul(seg, ones_row[0:1, 0:P],
                                 bv_r, start=False, stop=True)
            gsb = sc * (SC // P)
            dstv = vA[:, gsb:gsb + 4, :].rearrange(
                "p s (h c) -> p s h c", h=HG)[:, :, :, :DK]
            srcv = ps[:].rearrange("p (s h c) -> p s h c", s=4, h=HG)
            nc.scalar.activation(dstv, srcv, AF.Copy)

    def proj_chunk(x_dram, sc, which):
        proj_mm(proj_dma(x_dram, sc), sc, which)

    # K, Q chunk 0 first (unblocks the first ST pair), then V weights +
    # chunk 0 (needed one pair later by the lagged PVs), then Wo. The
    # remaining K/V chunks interleave into the first attention block.
    proj_chunk(xkt, 0, "k")
    proj_chunk(xqt, 0, "q")
    dma_w(wvt, wv_s)
    proj_chunk(xvt, 0, "v")

    # ---------------------------------------------------- attention
    # Two levels of software pipelining against the in-order PE queue:
    #  - PV matmuls run one kv-pair behind their STs (never wait on abs/exp)
    #  - each block's normalization/out-projection tail is deferred into the
    #    middle of the NEXT block, so block boundaries don't stall DVE/ACT.
    pending_pv = []
    pending_tail = []

    def flush_pv():
        for mm in pending_pv:
            mm()
        pending_pv.clear()

    def flush_tail(n=None):
        take = len(pending_tail) if n is None else min(n, len(pending_tail))
        for f in pending_tail[:take]:
            f()
        del pending_tail[:take]

    cat_tiles = {}

    def get_cat(qc):
        if qc not in cat_tiles:
            cat_tiles[qc] = catp.tile([P, 2, QC], F32R, tag="cat", name=f"cat{qc}")
        return cat_tiles[qc]

    def norm_a(qc, pr, opt):
        catT = get_cat(qc)
        # 1/Z via exp(-ln(Z)) on ACT: certified f32r writes, PSUM-direct read
        lnz = zp.tile([1, 2 * QC], F32, tag="z")
        nc.scalar.activation(
            lnz, opt[:].rearrange("p (h f) -> p h f", h=2)[64:65, :, :], AF.Ln)
        rec = zp.tile([1, 2 * QC], BF16, tag="rec")
        nc.scalar.activation(rec, lnz, AF.Exp, scale=-1.0)
        for half in range(2):
            csl = slice(half * DK, (half + 1) * DK)
            dst = catT[csl, pr, :]
            nc.scalar.activation(
                dst, opt[0:DK, half * QC:(half + 1) * QC], AF.Copy)
        return rec

    def norm_b(qc, pr, rec):
        catT = get_cat(qc)
        bc2 = opp.tile([P, 2 * QC], F32, tag="o", name="bc")
        for half in range(2):
            csl = slice(half * DK, (half + 1) * DK)
            bc = bc2[csl, half * QC:(half + 1) * QC]
            nc.tensor.matmul(bc, ones_bcr,
                             rec[0:1, half * QC:(half + 1) * QC],
                             start=True, stop=True,
                             tile_position=(0, half * DK))
            dst = catT[csl, pr, :]
            nc.vector.tensor_tensor(dst, dst, bc.bitcast(F32R), ALU.mult)

    def norm(qc, pr, opt):
        norm_b(qc, pr, norm_a(qc, pr, opt))

    def outproj(qc, js):
        catT = get_cat(qc)
        for j in js:
            sb = qc * (QC // P) + j
            o_t = otp.tile([P, D], F32, tag="ot")
            po = stp.tile([P, 2 * QC], F32, tag="st", name="po")
            for nn in range(2):
                seg = po[:, nn * QC:(nn + 1) * QC]
                for kc in range(2):
                    nc.tensor.matmul(
                        seg, catT[:, kc, j * P:(j + 1) * P],
                        wo_s[:, kc, nn * QC:(nn + 1) * QC],
                        start=(kc == 0), stop=(kc == 1))
            if j % 2 == 0:
                nc.scalar.activation(o_t, po, AF.Copy)
            else:
                nc.vector.tensor_copy(o_t, po)
            nc.sync.dma_start(out[sb * P:(sb + 1) * P, :], o_t)

    def attn_pair(qc, pr, pv, opt):
        qsl = slice(qc * QC, (qc + 1) * QC)
        hA, hB = 2 * pr, 2 * pr + 1
        pa_t = pap.tile([P, 2 * 2 * QC], F32R, tag="pa")
        sabs = sap.tile([P, 2 * 2 * QC], F32, tag="sa")
        for i in range(2):
            kv = 2 * pv + i
            ksl = slice(kv * P, (kv + 1) * P)
            st = stp.tile([P, 2 * QC], F32, tag="st", name="stt")
            nc.tensor.matmul(
                st[:, :QC], kT[0:DK, pr, ksl],
                qT[0:DK, pr, qsl], start=True, stop=True,
                tile_position=(0, 0))
            nc.tensor.matmul(
                st[:, QC:], kT[DK:P, pr, ksl],
                qT[DK:P, pr, qsl], start=True, stop=True,
                tile_position=(DK, 0))
            score_exit(sabs[:, i * 2 * QC:(i + 1) * 2 * QC], st)
            nc.scalar.activation(pa_t[:, i * 2 * QC:(i + 1) * 2 * QC],
                                 sabs[:, i * 2 * QC:(i + 1) * 2 * QC],
                                 AF.Exp, scale=SCALE)

            def mk_pv(kv=kv, off=i * 2 * QC, pa_t=pa_t, opt=opt,
                      hA=hA, hB=hB):
                nc.tensor.matmul(
                    opt[0:DK + 1, :QC],
                    vA[:, kv, hA * 65:hA * 65 + 65],
                    pa_t[:, off:off + QC],
                    start=(kv == 0), stop=(kv == KVB - 1))
                nc.tensor.matmul(
                    opt[0:DK + 1, QC:],
                    vA[:, kv, hB * 65:hB * 65 + 65],
                    pa_t[:, off + QC:off + 2 * QC],
                    start=(kv == 0), stop=(kv == KVB - 1))
            if i == 0:
                flush_pv()
            pending_pv.append(mk_pv)

    # --- qc 0: pr0/pr1 pair streams merged so both head-pairs' abs/exp work
    # rides each arriving K/V chunk (DMA otherwise starves DVE/ACT here).
    opt0 = opp.tile([P, 2 * QC], F32, tag="o")
    opt1 = opp.tile([P, 2 * QC], F32, tag="o")
    kv_tiles = {}
    xq1_tile = None
    for pv in range(KVB // 2):
        # prefetch chunk DMAs one pair ahead of their matmuls (xs bufs=3)
        if pv in (0, 2, 4):
            sc = pv // 2 + 1
            kv_tiles[sc] = [proj_dma(xkt, sc)]
        if pv in (1, 3, 5):
            sc = (pv + 1) // 2
            kv_tiles[sc].append(proj_dma(xvt, sc))
        if pv in (2, 4, 6):
            sc = pv // 2
            flush_pv()
            xk_t, xv_t = kv_tiles.pop(sc)
            proj_mm(xk_t, sc, "k")
            proj_mm(xv_t, sc, "v")
        attn_pair(0, 0, pv, opt0)
        if pv == KVB // 2 - 1:
            flush_pv()
            norm(0, 0, opt0)
        attn_pair(0, 1, pv, opt1)
        if pv == 5:
            nc.sync.dma_start(
                wo_s, wot.bitcast(F32R).rearrange("(kc p) d -> p kc d", p=P))
        if pv == 6:
            xq1_tile = proj_dma(xqt, 1)
        if pv == 7:
            proj_mm(xq1_tile, 1, "q")
    flush_pv()
    pending_tail.append(lambda: norm(0, 1, opt1))
    pending_tail.append(lambda: outproj(0, (0, 1)))
    pending_tail.append(lambda: outproj(0, (2, 3)))

    # --- qc 1..3: per-(qc, pr) blocks with deferred tails
    for qc in range(1, S // QC):
        for pr in range(2):
            opt = opp.tile([P, 2 * QC], F32, tag="o")
            for pv in range(KVB // 2):
                attn_pair(qc, pr, pv, opt)
                if pv >= 1:
                    flush_tail(1)
            if pr == 0:
                box = {}

                def t_na(qc=qc, pr=pr, opt=opt, box=box):
                    box["rec"] = norm_a(qc, pr, opt)

                def t_nb(qc=qc, pr=pr, box=box):
                    norm_b(qc, pr, box["rec"])
                pending_tail.append(t_na)
                pending_tail.append(t_nb)
                if qc + 1 < S // QC:
                    pending_tail.append(
                        lambda qc=qc: proj_chunk(xqt, qc + 1, "q"))
            else:
                box = {}

                def t_na(qc=qc, pr=pr, opt=opt, box=box):
                    box["rec"] = norm_a(qc, pr, opt)

                def t_nb(qc=qc, pr=pr, box=box):
                    norm_b(qc, pr, box["rec"])
                pending_tail.append(t_na)
                pending_tail.append(t_nb)
                for j in range(4):
                    pending_tail.append(lambda qc=qc, j=j: outproj(qc, (j,)))
    flush_pv()
    flush_tail()


_NC_CACHE = {}


def _get_nc():
    if "nc" not in _NC_CACHE:
        _NC_CACHE["nc"] = build_nc()
    return _NC_CACHE["nc"]


def make_in_maps(q, k, v, Wq, bq, Wk, bk, Wv, bv, Wo, bo):
    import ml_dtypes
    bf16 = ml_dtypes.bfloat16
    xT = [np.ascontiguousarray(np.asarray(a, np.float32).T.astype(bf16))
          for a in (q[0], k[0], v[0], q[1], k[1], v[1])]
    in_maps = []
    for c in range(N_CORES):
        b, g = divmod(c, 4)
        sl = slice(g * GC, (g + 1) * GC)
        in_maps.append({
            "xqt": xT[3 * b + 0],
            "xkt": xT[3 * b + 1],
            "xvt": xT[3 * b + 2],
            "wqt": np.ascontiguousarray(Wq[sl].T.astype(bf16)),
            "wkt": np.ascontiguousarray(Wk[sl].T.astype(bf16)),
            "wvt": np.ascontiguousarray(Wv[sl].T.astype(bf16)),
            "wot": np.ascontiguousarray(Wo[:, sl].T),
            "bq": np.ascontiguousarray(bq[sl].astype(bf16)),
            "bk": np.ascontiguousarray(bk[sl].astype(bf16)),
            "bv": np.ascontiguousarray(bv[sl].astype(bf16)),
        })
    return in_maps


def kernel(q, k, v, Wq, bq, Wk, bk, Wv, bv, Wo, bo, _trace=False):
    from concourse.bass_utils import run_bass_kernel_spmd

    q, k, v = (np.asarray(a, np.float32) for a in (q, k, v))
    Wq, bq, Wk, bk, Wv, bv, Wo, bo = (
        np.asarray(a, np.float32) for a in (Wq, bq, Wk, bk, Wv, bv, Wo, bo))

    nc = _get_nc()
    in_maps = make_in_maps(q, k, v, Wq, bq, Wk, bk, Wv, bv, Wo, bo)
    res = run_bass_kernel_spmd(nc, in_maps, core_ids=list(range(N_CORES)),
                               trace=_trace)
    partials = np.stack([r["out"] for r in res.results])  # [8, S, D]
    full = partials.reshape(2, 4, S, D).sum(axis=1) + bo[None, None, :]
    if _trace:
        return full.astype(np.float32), res
    return full.astype(np.float32)


# revision 6
# speedup vs baseline: 1.0810x; 1.0068x over previous
"""Trainium2 Bass kernel for nn_MultiHeadAttention_70866960384614.

MHA: B=2, S=2048, D_MODEL=1024, HEADS=16, D_K=64, softmax(|QK^T|/8) @ V.

Sharding (8 cores): data-parallel over batch (2) x tensor-parallel over head
groups (4 groups of 4 heads). Host pre-transposes x and the weight slices so
the device does ZERO layout transposes: per core the inputs are
  xqT/xkT/xvT [1024, 2048]  (x^T, fp32)
  wqT/wkT/wvT [1024, 256]   (W[group].T)
  woT         [256, 1024]   (Wo[:, group].T)
DMA feeds matmul operands directly as float32r (full-rate fp32 on the PE).

Per-core dataflow:
  QT = Wq @ xT (+bq)            [256, 2048]   (= wqT.T @ xqT)
  KT likewise; V = x @ WvT + bv  stored kv-major with a ones column per head
  per (qchunk, head-pair): for each kv pair:
     ST = K Q^T (2 heads packed in the PE array), DVE/ACT |.| exit to SBUF,
     one bulk ACT exp over [128, 2048], PV accumulates [V|1].T @ P
  normalization: Z rows -> DVE reciprocal -> PE broadcast -> DVE multiply
  out partial = catT.T @ WoT -> DMA (host sums the 4 group partials + bo)
"""

import json
from contextlib import ExitStack

import numpy as np

import concourse.bass as bass
import concourse.mybir as mybir
import concourse.tile as tile
from concourse.vector_clock import ScopedClock

F32 = mybir.dt.float32
F32R = mybir.dt.float32r
BF16 = mybir.dt.bfloat16
AF = mybir.ActivationFunctionType
ALU = mybir.AluOpType

S = 2048
D = 1024
HG = 4            # heads per core
DK = 64
GC = HG * DK      # 256
P = 128
N_CORES = 8
SC = 512          # s-chunk for projection streaming
QC = 512          # q-chunk within attention
KVB = S // P      # 16 kv blocks
SCALE = 0.125

# every ACT_EXIT_EVERYth score-exit goes to ACT instead of DVE (engine balance)
ACT_EXIT_EVERY = 0   # 0 = never


class TileContextCompat(tile.TileContext):
    """This container's walrus build rejects >1 sync-wait on a CTRL (Drain)
    instruction; spread the kernel-tail DMA-lane waits across one drain
    each instead of piling them on a single drain."""

    def _drain_and_barrier(self, tick_clock, wait_clock):
        drain_inst = self.nc.sync.drain()
        wait_clock.add_sem_waits(
            drain_inst.ins, ScopedClock({None: tick_clock.global_clock}))
        si = drain_inst.ins.sync_info
        extra = []
        if si is not None and si.on_wait is not None:
            while len(si.on_wait) > 1:
                extra.append(si.on_wait.pop())
        for w in extra:
            d2 = self.nc.sync.drain()
            if d2.ins.sync_info is None:
                d2.ins.sync_info = mybir.SyncInfo(on_wait=[w], on_update=[])
            else:
                d2.ins.sync_info.on_wait.append(w)
        self.nc.all_engine_barrier()
        assert self.sems is not None
        popped = self.nc._tile_sem_poison_stack.pop()
        assert popped is self._sem_poison
        self.nc.clear_and_free_semaphores(list(self.sems.allocated().values()))
        self.nc.all_engine_barrier()


def build_nc():
    nc = bass.Bass("TRN2", target_bir_lowering=False, debug=False,
                   num_devices=N_CORES)

    xqt = nc.dram_tensor("xqt", [D, S], BF16, kind="ExternalInput").ap()
    xkt = nc.dram_tensor("xkt", [D, S], BF16, kind="ExternalInput").ap()
    xvt = nc.dram_tensor("xvt", [D, S], BF16, kind="ExternalInput").ap()
    wqt = nc.dram_tensor("wqt", [D, GC], BF16, kind="ExternalInput").ap()
    wkt = nc.dram_tensor("wkt", [D, GC], BF16, kind="ExternalInput").ap()
    wvt = nc.dram_tensor("wvt", [D, GC], BF16, kind="ExternalInput").ap()
    wot = nc.dram_tensor("wot", [GC, D], F32, kind="ExternalInput").ap()
    bq = nc.dram_tensor("bq", [GC], BF16, kind="ExternalInput").ap()
    bk = nc.dram_tensor("bk", [GC], BF16, kind="ExternalInput").ap()
    out = nc.dram_tensor("out", [S, D], F32, kind="ExternalOutput").ap()

    with ExitStack() as ctx:
        tc = ctx.enter_context(TileContextCompat(nc))
        _emit(ctx, tc, xqt, xkt, xvt, wqt, wkt, wvt, wot, bq, bk, out)

    fixed = _split_multi_waits(nc.to_json_bytes())
    nc.to_json_bytes = lambda: fixed
    return nc


def _split_multi_waits(raw):
    """Walrus here accepts only one sync-wait per instruction; hoist extras
    onto wait-only EventSemaphore instructions on the same engine."""
    m = json.loads(raw)
    counter = [0]

    def fix_block(b):
        new = []
        for inst in b.get("instructions", []):
            si = inst.get("sync_info")
            if si and si.get("on_wait") and len(si["on_wait"]) > 1:
                waits = si["on_wait"]
                for w in waits[:-1]:
                    counter[0] += 1
                    new.append({
                        "debug": inst.get("debug", 0),
                        "engine": inst["engine"],
                        "ins": [],
                        "outs": [],
                        "name": f"I-wsplit-{counter[0]}",
                        "opcode": "EventSemaphore",
                        "sync_info": {"on_update": [], "on_wait": [w]},
                    })
                si["on_wait"] = waits[-1:]
            new.append(inst)
        b["instructions"] = new
        for sub in b.get("blocks", []):
            fix_block(sub)

    for fn in m["functions"]:
        for b in fn.get("blocks", []):
            fix_block(b)
    return json.dumps(m).encode()


def _emit(ctx, tc, xqt, xkt, xvt, wqt, wkt, wvt, wot, bq, bk, out):
    nc = tc.nc

    persist = ctx.enter_context(tc.tile_pool(name="persist", bufs=1))
    xs = ctx.enter_context(tc.tile_pool(name="xs", bufs=3))
    stp = ctx.enter_context(tc.tile_pool(name="st", bufs=2, space="PSUM"))
    opp = ctx.enter_context(tc.tile_pool(name="op", bufs=2, space="PSUM"))
    pap = ctx.enter_context(tc.tile_pool(name="pa", bufs=3))
    sap = ctx.enter_context(tc.tile_pool(name="sa", bufs=3))
    zp = ctx.enter_context(tc.tile_pool(name="zp", bufs=2))
    otp = ctx.enter_context(tc.tile_pool(name="ot", bufs=4))
    catp = ctx.enter_context(tc.tile_pool(name="cat", bufs=2))

    # ------------------------------------------------------- persistent
    qT = persist.tile([P, 2, S], F32R)
    kT = persist.tile([P, 2, S], F32R)
    vA = persist.tile([P, KVB, HG * (DK + 1)], F32R)
    wq_s = persist.tile([P, D // P, GC], BF16)
    wk_s = persist.tile([P, D // P, GC], BF16)
    wv_s = persist.tile([P, D // P, GC], BF16)
    wo_s = persist.tile([P, GC // P, D], F32R)
    bq_r = persist.tile([1, GC], BF16)
    bk_r = persist.tile([1, GC], BF16)

    # ones rows (memset + DVE self-copy so they count as f32r-produced)
    ones_row = persist.tile([1, QC], BF16)
    nc.vector.memset(ones_row, 1.0)
    ones_bcr = persist.tile([1, DK], BF16)
    nc.vector.memset(ones_bcr, 1.0)
    # V ones columns
    nc.vector.memset(vA.bitcast(F32), 1.0)
    ones_cols = vA[:].rearrange("p s (h c) -> p s h c", h=HG)[:, :, :, DK:]
    nc.vector.tensor_copy(ones_cols, ones_cols.bitcast(F32))

    # ------------------------------------------------------ weight DMAs
    # (K and Q weights first: the first ST pair needs only kT/qT chunk 0;
    # V weights and chunks follow just in time for the lagged PV matmuls.)
    def dma_w(w_dram, w_sb):
        nc.sync.dma_start(
            w_sb, w_dram.rearrange("(kc p) c -> p kc c", p=P))

    dma_w(wkt, wk_s)
    dma_w(wqt, wq_s)
    nc.gpsimd.dma_start(bq_r, bq[None, :])
    nc.gpsimd.dma_start(bk_r, bk[None, :])

    # PE warmup: a continuous chain of tiny matmuls spans the initial DMA
    # window so the PE p-state is fully ramped when the projections start.
    wup = stp.tile([P, 2 * QC], F32, tag="st", name="wup")
    for _ in range(150):
        nc.tensor.matmul(wup[0:1, 0:DK], ones_row[0:1, 0:1],
                         ones_row[0:1, 0:DK], start=True, stop=True)

    exit_ctr = [0]

    def score_exit(dst, src):
        exit_ctr[0] += 1
        if ACT_EXIT_EVERY and exit_ctr[0] % ACT_EXIT_EVERY == 0:
            nc.scalar.activation(dst, src, AF.Abs)
        else:
            du, su = dst.bitcast(mybir.dt.uint32), src.bitcast(mybir.dt.uint32)
            nc.vector.tensor_scalar(du, su, 0x7FFFFFFF, None, ALU.bitwise_and)

    def proj_dma(x_dram, sc, split=False):
        xt = xs.tile([P, D // P, SC], BF16, tag="xs")
        src = x_dram.rearrange("(kc p) s -> p kc s", p=P)[
            :, :, sc * SC:(sc + 1) * SC]
        if split:
            # per-kc DMAs: accumulation matmuls start as each slice lands
            for kc in range(D // P):
                nc.sync.dma_start(xt[:, kc:kc + 1, :], src[:, kc:kc + 1, :])
        else:
            nc.sync.dma_start(xt, src)
        return xt

    def proj_mm(xt, sc, which):
        """Matmuls+exit for one SC-chunk of a projection. which: 'k'|'v'|'q'."""
        ps = stp.tile([P, 2 * QC], F32, tag="st", name="pj")
        if which in ("k", "q"):
            w_sb, dstT, b_r = ((wk_s, kT, bk_r) if which == "k"
                               else (wq_s, qT, bq_r))
            for m in range(2):
                half = ps[:, m * SC:(m + 1) * SC]
                for kc in range(D // P):
                    nc.tensor.matmul(
                        half, w_sb[:, kc, m * P:(m + 1) * P],
                        xt[:, kc, :], start=(kc == 0), stop=False)
                nc.tensor.matmul(
                    half, b_r[0:1, m * P:(m + 1) * P],
                    ones_row, start=False, stop=True)
            dst = dstT[:, :, sc * SC:(sc + 1) * SC]
            nc.scalar.activation(
                dst, ps[:].rearrange("p (m f) -> p m f", m=2), AF.Copy)
        else:
            for sb in range(SC // P):
                seg = ps[:, sb * GC:(sb + 1) * GC]
                for kc in range(D // P):
                    nc.tensor.matmul(
                        seg, xt[:, kc, sb * P:(sb + 1) * P],
                        wv_s[:, kc, :], start=(kc == 0),
                        stop=(kc == D // P - 1))
            gsb = sc * (SC // P)
            dstv = vA[:, gsb:gsb + 4, :].rearrange(
                "p s (h c) -> p s h c", h=HG)[:, :, :, :DK]
            srcv = ps[:].rearrange("p (s h c) -> p s h c", s=4, h=HG)
            nc.scalar.activation(dstv, srcv, AF.Copy)

    def proj_chunk(x_dram, sc, which):
        proj_mm(proj_dma(x_dram, sc), sc, which)

    # K, Q chunk 0 first (unblocks the first ST pair), then V weights +
    # chunk 0 (needed one pair later by the lagged PVs), then Wo. The
    # remaining K/V chunks interleave into the first attention block.
    proj_chunk(xkt, 0, "k")
    proj_chunk(xqt, 0, "q")
    dma_w(wvt, wv_s)
    proj_chunk(xvt, 0, "v")

    # ---------------------------------------------------- attention
    # Two levels of software pipelining against the in-order PE queue:
    #  - PV matmuls run one kv-pair behind their STs (never wait on abs/exp)
    #  - each block's normalization/out-projection tail is deferred into the
    #    middle of the NEXT block, so block boundaries don't stall DVE/ACT.
    pending_pv = []
    pending_tail = []

    def flush_pv():
        for mm in pending_pv:
            mm()
        pending_pv.clear()

    def flush_tail(n=None):
        take = len(pending_tail) if n is None else min(n, len(pending_tail))
        for f in pending_tail[:take]:
            f()
        del pending_tail[:take]

    cat_tiles = {}

    def get_cat(qc):
        if qc not in cat_tiles:
            cat_tiles[qc] = catp.tile([P, 2, QC], F32R, tag="cat", name=f"cat{qc}")
        return cat_tiles[qc]

    def norm_a(qc, pr, opt):
        catT = get_cat(qc)
        # 1/Z via exp(-ln(Z)) on ACT: certified f32r writes, PSUM-direct read
        lnz = zp.tile([1, 2 * QC], F32, tag="z")
        nc.scalar.activation(
            lnz, opt[:].rearrange("p (h f) -> p h f", h=2)[64:65, :, :], AF.Ln)
        rec = zp.tile([1, 2 * QC], BF16, tag="rec")
        nc.scalar.activation(rec, lnz, AF.Exp, scale=-1.0)
        for half in range(2):
            csl = slice(half * DK, (half + 1) * DK)
            dst = catT[csl, pr, :]
            nc.scalar.activation(
                dst, opt[0:DK, half * QC:(half + 1) * QC], AF.Copy)
        return rec

    def norm_b(qc, pr, rec):
        catT = get_cat(qc)
        bc2 = opp.tile([P, 2 * QC], F32, tag="o", name="bc")
        for half in range(2):
            csl = slice(half * DK, (half + 1) * DK)
            bc = bc2[csl, half * QC:(half + 1) * QC]
            nc.tensor.matmul(bc, ones_bcr,
                             rec[0:1, half * QC:(half + 1) * QC],
                             start=True, stop=True,
                             tile_position=(0, half * DK))
            dst = catT[csl, pr, :]
            nc.vector.tensor_tensor(dst, dst, bc.bitcast(F32R), ALU.mult)

    def norm(qc, pr, opt):
        norm_b(qc, pr, norm_a(qc, pr, opt))

    def outproj(qc, js):
        catT = get_cat(qc)
        for j in js:
            sb = qc * (QC // P) + j
            o_t = otp.tile([P, D], F32, tag="ot")
            po = stp.tile([P, 2 * QC], F32, tag="st", name="po")
            for nn in range(2):
                seg = po[:, nn * QC:(nn + 1) * QC]
                for kc in range(2):
                    nc.tensor.matmul(
                        seg, catT[:, kc, j * P:(j + 1) * P],
                        wo_s[:, kc, nn * QC:(nn + 1) * QC],
                        start=(kc == 0), stop=(kc == 1))
            if j % 2 == 0:
                nc.scalar.activation(o_t, po, AF.Copy)
            else:
                nc.vector.tensor_copy(o_t, po)
            nc.sync.dma_start(out[sb * P:(sb + 1) * P, :], o_t)

    def attn_pair(qc, pr, pv, opt):
        qsl = slice(qc * QC, (qc + 1) * QC)
        hA, hB = 2 * pr, 2 * pr + 1
        pa_t = pap.tile([P, 2 * 2 * QC], F32R, tag="pa")
        sabs = sap.tile([P, 2 * 2 * QC], F32, tag="sa")
        for i in range(2):
            kv = 2 * pv + i
            ksl = slice(kv * P, (kv + 1) * P)
            st = stp.tile([P, 2 * QC], F32, tag="st", name="stt")
            nc.tensor.matmul(
                st[:, :QC], kT[0:DK, pr, ksl],
                qT[0:DK, pr, qsl], start=True, stop=True,
                tile_position=(0, 0))
            nc.tensor.matmul(
                st[:, QC:], kT[DK:P, pr, ksl],
                qT[DK:P, pr, qsl], start=True, stop=True,
                tile_position=(DK, 0))
            score_exit(sabs[:, i * 2 * QC:(i + 1) * 2 * QC], st)
            nc.scalar.activation(pa_t[:, i * 2 * QC:(i + 1) * 2 * QC],
                                 sabs[:, i * 2 * QC:(i + 1) * 2 * QC],
                                 AF.Exp, scale=SCALE)

            def mk_pv(kv=kv, off=i * 2 * QC, pa_t=pa_t, opt=opt,
                      hA=hA, hB=hB):
                nc.tensor.matmul(
                    opt[0:DK + 1, :QC],
                    vA[:, kv, hA * 65:hA * 65 + 65],
                    pa_t[:, off:off + QC],
                    start=(kv == 0), stop=(kv == KVB - 1))
                nc.tensor.matmul(
                    opt[0:DK + 1, QC:],
                    vA[:, kv, hB * 65:hB * 65 + 65],
                    pa_t[:, off + QC:off + 2 * QC],
                    start=(kv == 0), stop=(kv == KVB - 1))
            if i == 0:
                flush_pv()
            pending_pv.append(mk_pv)

    # --- qc 0: pr0/pr1 pair streams merged so both head-pairs' abs/exp work
    # rides each arriving K/V chunk (DMA otherwise starves DVE/ACT here).
    opt0 = opp.tile([P, 2 * QC], F32, tag="o")
    opt1 = opp.tile([P, 2 * QC], F32, tag="o")
    kv_tiles = {}
    xq1_tile = None
    for pv in range(KVB // 2):
        # prefetch chunk DMAs one pair ahead of their matmuls (xs bufs=3)
        if pv in (0, 2, 4):
            sc = pv // 2 + 1
            kv_tiles[sc] = [proj_dma(xkt, sc)]
        if pv in (1, 3, 5):
            sc = (pv + 1) // 2
            kv_tiles[sc].append(proj_dma(xvt, sc))
        if pv in (2, 4, 6):
            sc = pv // 2
            flush_pv()
            xk_t, xv_t = kv_tiles.pop(sc)
            proj_mm(xk_t, sc, "k")
            proj_mm(xv_t, sc, "v")
        attn_pair(0, 0, pv, opt0)
        if pv == KVB // 2 - 1:
            flush_pv()
            norm(0, 0, opt0)
        attn_pair(0, 1, pv, opt1)
        if pv == 5:
            nc.sync.dma_start(
                wo_s, wot.bitcast(F32R).rearrange("(kc p) d -> p kc d", p=P))
        if pv == 6:
            xq1_tile = proj_dma(xqt, 1)
        if pv == 7:
            proj_mm(xq1_tile, 1, "q")
    flush_pv()
    pending_tail.append(lambda: norm(0, 1, opt1))
    pending_tail.append(lambda: outproj(0, (0, 1)))
    pending_tail.append(lambda: outproj(0, (2, 3)))

    # --- qc 1..3: per-(qc, pr) blocks with deferred tails
    for qc in range(1, S // QC):
        for pr in range(2):
            opt = opp.tile([P, 2 * QC], F32, tag="o")
            for pv in range(KVB // 2):
                attn_pair(qc, pr, pv, opt)
                if pv >= 1:
                    flush_tail(1)
            if pr == 0:
                box = {}

                def t_na(qc=qc, pr=pr, opt=opt, box=box):
                    box["rec"] = norm_a(qc, pr, opt)

                def t_nb(qc=qc, pr=pr, box=box):
                    norm_b(qc, pr, box["rec"])
                pending_tail.append(t_na)
                pending_tail.append(t_nb)
                if qc + 1 < S // QC:
                    pending_tail.append(
                        lambda qc=qc: proj_chunk(xqt, qc + 1, "q"))
            else:
                box = {}

                def t_na(qc=qc, pr=pr, opt=opt, box=box):
                    box["rec"] = norm_a(qc, pr, opt)

                def t_nb(qc=qc, pr=pr, box=box):
                    norm_b(qc, pr, box["rec"])
                pending_tail.append(t_na)
                pending_tail.append(t_nb)
                for j in range(4):
                    pending_tail.append(lambda qc=qc, j=j: outproj(qc, (j,)))
    flush_pv()
    flush_tail()


_NC_CACHE = {}


def _get_nc():
    if "nc" not in _NC_CACHE:
        _NC_CACHE["nc"] = build_nc()
    return _NC_CACHE["nc"]


def make_in_maps(q, k, v, Wq, bq, Wk, bk, Wv, bv, Wo, bo):
    import ml_dtypes
    bf16 = ml_dtypes.bfloat16
    xT = [np.ascontiguousarray(np.asarray(a, np.float32).T.astype(bf16))
          for a in (q[0], k[0], v[0], q[1], k[1], v[1])]
    in_maps = []
    for c in range(N_CORES):
        b, g = divmod(c, 4)
        sl = slice(g * GC, (g + 1) * GC)
        in_maps.append({
            "xqt": xT[3 * b + 0],
            "xkt": xT[3 * b + 1],
            "xvt": xT[3 * b + 2],
            "wqt": np.ascontiguousarray(Wq[sl].T.astype(bf16)),
            "wkt": np.ascontiguousarray(Wk[sl].T.astype(bf16)),
            "wvt": np.ascontiguousarray(Wv[sl].T.astype(bf16)),
            "wot": np.ascontiguousarray(Wo[:, sl].T),
            "bq": np.ascontiguousarray(bq[sl].astype(bf16)),
            "bk": np.ascontiguousarray(bk[sl].astype(bf16)),
        })
    return in_maps


def kernel(q, k, v, Wq, bq, Wk, bk, Wv, bv, Wo, bo, _trace=False):
    from concourse.bass_utils import run_bass_kernel_spmd

    q, k, v = (np.asarray(a, np.float32) for a in (q, k, v))
    Wq, bq, Wk, bk, Wv, bv, Wo, bo = (
        np.asarray(a, np.float32) for a in (Wq, bq, Wk, bk, Wv, bv, Wo, bo))

    nc = _get_nc()
    in_maps = make_in_maps(q, k, v, Wq, bq, Wk, bk, Wv, bv, Wo, bo)
    res = run_bass_kernel_spmd(nc, in_maps, core_ids=list(range(N_CORES)),
                               trace=_trace)
    partials = np.stack([r["out"] for r in res.results])  # [8, S, D]
    # softmax rows sum to 1, so the V bias passes through attention exactly:
    # out += Wo @ bv (folded here) + bo
    bias = (Wo @ bv + bo).astype(np.float32)
    full = partials.reshape(2, 4, S, D).sum(axis=1) + bias[None, None, :]
    if _trace:
        return full.astype(np.float32), res
    return full.astype(np.float32)
